# revision 18
# baseline (speedup 1.0000x reference)
# Bloom parallel attention block on 8 trn2 NeuronCores, tensor-parallel over
# heads (2 heads per core).  Feature-major layouts throughout.
#
# fp8 (e4m3, max 240) + DoubleRow variant: QKV, ctx, softmax-denominator and
# dense matmuls run as K=256 fp8 DoubleRow matmuls (2x PE throughput); the
# score matmuls stay bf16 (K=128, no DR gain).  Power-of-2 scales keep fp8
# operands out of the subnormal range:
#   wqkv, wdense scaled 2^10 on host; descale folded into the QKV psum
#   evacuation (tensor_scalar mult) and the host-side output descale.
#   v' = v * exp(alibi) * 2^4,  ones' = exp(alibi) * 2^-1  =>  ctx = 2^5 ctx.
#   probs = exp(scores - 5.5) via ACT bias (global shift cancels in the
#   softmax normalization; keeps exp <= 240 for fp8e4).
#   dense out = 2^(5+10) * true; residual pre-scaled 2^15 on host, output
#   descaled 2^-15 on host.
# ctx is gathered in fp8 (halves collective bytes vs bf16).
#
# Per core r (heads 2r, 2r+1):
#   QKV DR matmul -> Q^T/K^T [d, s] bf16 and V^T [d, s] bf16 per batch.
#   V^T is PE-transposed to V [s, d] and scaled by exp(alibi[k])*2^4 into
#   fp8; ones' tiles = exp(alibi[k])*2^-1 broadcast columns (fp8).
#   attention (per b, head hl, 512-wide q-chunk qc), scores transposed [k, q]:
#     scores^T = K^T_tile.T @ Q^T       x2 k-tiles  (PE bf16, fp32 psum)
#     exp(scores - 5.5) over both k-tiles (ACT, bf16 out)
#     * mask01^T                        (DVE, 0/1 mask, fp8e4 out)
#     ctx^T += V'_pair.T @ probs        (PE fp8 DoubleRow, 4 k-tile pairs..)
#     sum   += ones'_pair.T @ probs     (PE fp8 DoubleRow, accum 8 pairs)
#     ctx^T *= 1/sum -> fp8 -> DMA to cc chunk
#   Pipelining: QKV(b1) matmuls are interleaved into attention(b0) k-loops,
#   dense matmuls into attention(b1) k-loops, so the PE never idles.  ctx
#   is AllGathered in 6 fp8 column chunks (2 for b0, 4 for b1).
#   dense: out^T[o_local, s] = wdT_pair.T @ ctx^T_full (fp8 DR) + resid'
#     (column-parallel => no all-reduce; host concatenates output slices)
import os
import sys

import numpy as np

if "/opt/trn_rl_repo" not in sys.path:
    sys.path.insert(0, "/opt/trn_rl_repo")

import ml_dtypes

import concourse.bass as bass
import concourse.mybir as mybir
import concourse.tile as tile
from concourse import bacc, bass_utils

B, S, H, NH = 2, 2048, 2048, 16
HD = H // NH            # 128
NCORES = 8
HPC = NH // NCORES      # heads per core = 2
OSH = 3 * H // NCORES   # qkv output rows per core = 768
DSH = H // NCORES       # dense output cols per core = 256
P = 128
F32 = mybir.dt.float32
BF16 = mybir.dt.bfloat16
F8 = mybir.dt.float8e4
AF = mybir.ActivationFunctionType
DR = mybir.MatmulPerfMode.DoubleRow
NPBF16 = ml_dtypes.bfloat16
NPF8 = ml_dtypes.float8_e4m3

WSCALE = 10          # wqkv/wdense host scale 2^10
VSCALE = 4           # v' = v*e*2^4
OWSCALE = -1         # ones' = e*2^-1  => ctxn = 2^5 ctx
ESHIFT = 5.5         # probs = exp(s - 5.5); cancels in normalization
OUT_DESCALE = np.float32(2.0 ** -(5 + WSCALE))   # host output descale
INV = np.float32(1.0 / np.sqrt(HD))


def build_nc():
    nc = bacc.Bacc(
        "TRN2",
        target_bir_lowering=False,
        debug=False,
        num_devices=NCORES,
    )

    hidT = nc.dram_tensor("hidT", [H, B * S], F8, kind="ExternalInput").ap()
    wqkvT = nc.dram_tensor("wqkvT", [H, OSH], F8, kind="ExternalInput").ap()
    bqkv = nc.dram_tensor("bqkv", [P, 6], F32, kind="ExternalInput").ap()
    mask01T = nc.dram_tensor("mask01T", [P, 4 * 16 * 512], BF16, kind="ExternalInput").ap()
    alibi_e = nc.dram_tensor("alibi_e", [P, 2 * 2 * HPC * 16], F32, kind="ExternalInput").ap()
    alibi8 = nc.dram_tensor("alibi8", [P, 2 * HPC * 16], F8, kind="ExternalInput").ap()
    wdT = nc.dram_tensor("wdT", [H, DSH], F8, kind="ExternalInput").ap()
    residT = nc.dram_tensor("residT", [DSH, B * S], F32, kind="ExternalInput").ap()
    ones = nc.dram_tensor("ones", [P, P], BF16, kind="ExternalInput").ap()
    eye = nc.dram_tensor("eye", [P, P], BF16, kind="ExternalInput").ap()
    outT = nc.dram_tensor("outT", [DSH, B * S], F32, kind="ExternalOutput").ap()

    with tile.TileContext(nc) as tc:
        ccg = [list(range(NCORES))]
        with (
            tc.tile_pool(name="const", bufs=1) as constp,
            tc.tile_pool(name="dram", bufs=1, space="DRAM") as dramp,
        ):
            bq_sb = constp.tile([P, 6], F32)
            nc.gpsimd.dma_start(bq_sb, bqkv)
            ale_sb = constp.tile([P, 2 * 2 * HPC * 16], F32)
            nc.gpsimd.dma_start(ale_sb, alibi_e)
            # fp8 exp(alibi)*2^-1 columns: used directly (partition-broadcast
            # along the output dim) as the softmax-denominator matmul weights
            ale8_sb = constp.tile([P, 2 * HPC * 16], F8)
            nc.gpsimd.dma_start(ale8_sb, alibi8)
            ones_sb = constp.tile(
                [P, P], BF16,
                name="ones_sb_ldw" if os.environ.get("BASS_LDW_OPT") else "ones_sb",
            )
            nc.gpsimd.dma_start(ones_sb, ones)
            eye_sb = constp.tile([P, P], BF16)
            nc.gpsimd.dma_start(eye_sb, eye)
            esh_sb = constp.tile([P, 1], F32)
            nc.gpsimd.memset(esh_sb, -ESHIFT)

            # ctx gather chunks (fp8): both batches in 4 column quarters,
            # each fired as soon as its q-chunk's two heads finish
            cc_spec = [(4, S // 4), (4, S // 4)]
            cc_in = [
                [
                    dramp.tile([HPC * HD, w], F8, name=f"cc_in{b}{i}")
                    for i in range(n)
                ]
                for b, (n, w) in enumerate(cc_spec)
            ]
            cc_out = [
                [
                    dramp.tile([H, w], F8, addr_space="Shared", name=f"cc_out{b}{i}")
                    for i in range(n)
                ]
                for b, (n, w) in enumerate(cc_spec)
            ]
            # b1 qc3 is gathered per-head so the first half fires one block
            # earlier and the tail gather is half-size.  Gathered row order
            # becomes [all even heads | all odd heads]; the tail dense chunk
            # uses a correspondingly permuted copy of wd.
            cc_in13 = [dramp.tile([P, 512], F8, name=f"cc_in13h{h}") for h in range(2)]
            cc_out13 = [
                dramp.tile([NCORES * P, 512], F8, addr_space="Shared", name=f"cc_out13h{h}")
                for h in range(2)
            ]

            def dma_ctx(b, qc, hl, ctxn_t):
                if b == 1 and qc == 3:
                    nc.sync.dma_start(cc_in13[hl], ctxn_t)
                    return
                n, w = cc_spec[b]
                chunk, qq = divmod(qc, 4 // n)
                nc.sync.dma_start(
                    cc_in[b][chunk][hl * P : (hl + 1) * P, qq * 512 : (qq + 1) * 512],
                    ctxn_t,
                )

            def all_gather(b, chunk):
                nc.gpsimd.collective_compute(
                    "AllGather", mybir.AluOpType.bypass, replica_groups=ccg,
                    ins=[cc_in[b][chunk].opt()], outs=[cc_out[b][chunk].opt()],
                )

            def all_gather13(h):
                nc.gpsimd.collective_compute(
                    "AllGather", mybir.AluOpType.bypass, replica_groups=ccg,
                    ins=[cc_in13[h].opt()], outs=[cc_out13[h].opt()],
                )

            with (
                tc.tile_pool(name="mask", bufs=1) as maskp,
                tc.tile_pool(name="qk1", bufs=1) as qk1p,
                tc.tile_pool(name="vt", bufs=1) as vtp,
                tc.tile_pool(name="v1", bufs=1) as v1p,
                tc.tile_pool(name="ow1", bufs=1) as ow1p,
            ):
                mask_sb = maskp.tile([P, 4, 16, 512], BF16)
                qk_sbs = [None, qk1p.tile([P, 2 * HPC, S], BF16, name="qksb1")]
                v_sbs = [None, v1p.tile([P, HPC, 16, P], F8, name="vsb1")]
                ow_sbs = [None, ow1p.tile([P, HPC, 16, P], F8, name="owsb1")]

                def attn_block(b, qc, hl, aps, attp, extra_mm):
                    """Attention for (b, head hl, q-chunk qc), k-tiles in
                    pairs; extra_mm(kp) emits extra matmuls per pair to keep
                    the PE busy while ACT/DVE run."""
                    qk = qk_sbs[b]
                    ctx_ps = aps.tile([P, 512], F32, tag="ctx", bufs=1)
                    sum_ps = aps.tile([P, 512], F32, tag="sum", bufs=1)
                    for kp in range(8):
                        kt0 = 2 * kp
                        s_ps = aps.tile([P, 1024], F32, tag="sco", bufs=2)
                        for u in range(2):
                            nc.tensor.matmul(
                                s_ps[:, u * 512 : (u + 1) * 512],
                                lhsT=qk[:, hl * 2 + 1, (kt0 + u) * P : (kt0 + u + 1) * P],
                                rhs=qk[:, hl * 2, qc * 512 : (qc + 1) * 512],
                                start=True,
                                stop=True,
                            )
                        exp_t = attp.tile([P, 1024], BF16, tag="exp")
                        nc.scalar.activation(exp_t, s_ps, AF.Exp, bias=esh_sb)
                        prob_t = attp.tile([P, 2, 512], F8, tag="prob")
                        nc.vector.tensor_mul(
                            prob_t,
                            exp_t.rearrange("p (u q) -> p u q", u=2),
                            mask_sb[:, qc, kt0 : kt0 + 2, :],
                        )
                        # weave BEFORE ctx/sum: the PE queue is FIFO, so the
                        # independent woven matmuls must sit ahead of the
                        # prob-dependent ones to fill the exp/prob latency
                        extra_mm(kp)
                        nc.tensor.matmul(
                            ctx_ps,
                            lhsT=v_sbs[b][:, hl, kt0 : kt0 + 2, :],
                            rhs=prob_t,
                            start=(kp == 0),
                            stop=(kp == 7),
                            perf_mode=DR,
                        )
                        nc.tensor.matmul(
                            sum_ps,
                            lhsT=ow_sbs[b][:, hl, kt0 : kt0 + 2, :],
                            rhs=prob_t,
                            start=(kp == 0),
                            stop=(kp == 7),
                            perf_mode=DR,
                        )
                    rec_t = attp.tile([P, 512], F32, tag="rec", bufs=2)
                    nc.vector.reciprocal_approx_fast(rec_t, sum_ps)
                    ctxn_t = attp.tile([P, 512], F8, tag="ctxn", bufs=2)
                    nc.vector.tensor_mul(ctxn_t, ctx_ps, rec_t)
                    dma_ctx(b, qc, hl, ctxn_t)

                # ---------- phase 1: QKV(b0), standalone ----------
                with (
                    tc.tile_pool(name="qk0", bufs=1) as qk0p,
                    tc.tile_pool(name="v0", bufs=1) as v0p,
                    tc.tile_pool(name="ow0", bufs=1) as ow0p,
                    tc.tile_pool(name="wq", bufs=1) as wqp,
                    tc.tile_pool(name="hid", bufs=16) as hidp,
                    tc.tile_pool(name="qps", bufs=3, space="PSUM") as qps,
                ):
                    qk_sbs[0] = qk0p.tile([P, 2 * HPC, S], BF16, name="qksb0")
                    v_sbs[0] = v0p.tile([P, HPC, 16, P], F8, name="vsb0")
                    ow_sbs[0] = ow0p.tile([P, HPC, 16, P], F8, name="owsb0")
                    wq_sb = wqp.tile([P, 16, OSH], F8)

                    def qkv_sc(b, sc, vT_sb):
                        """QKV for one 512-wide s-chunk: 6 o-tiles x 8
                        h-tile-pairs (fp8 DoubleRow); call emit(j) for j in
                        range(48).  V^T o-tiles are PE-transposed to V [k, d]
                        and scaled by exp(alibi[k])*2^4 into fp8; ones' tiles
                        built alongside at 2^-1."""
                        hid_ts = []
                        for hp in range(8):
                            if b == 0 and sc == 0:
                                nc.gpsimd.dma_start(
                                    wq_sb[:, 2 * hp : 2 * hp + 2, :],
                                    wqkvT[2 * hp * P : (2 * hp + 2) * P, :].rearrange(
                                        "(two p) o -> p two o", p=P
                                    ),
                                )
                            hid_t = hidp.tile([P, 2, 512], F8, tag="hid")
                            nc.sync.dma_start(
                                hid_t,
                                hidT[
                                    2 * hp * P : (2 * hp + 2) * P,
                                    b * S + sc * 512 : b * S + (sc + 1) * 512,
                                ].rearrange("(two p) s -> p two s", p=P),
                            )
                            hid_ts.append(hid_t)
                        state = {"ps": None}

                        def emit(j):
                            ot, hp = divmod(j, 8)
                            hl, t = divmod(ot, 3)
                            if hp == 0:
                                state["ps"] = qps.tile(
                                    [P, 512], F32, tag="qkvps", bufs=2,
                                    name=f"qps_{b}_{sc}_{ot}",
                                )
                            nc.tensor.matmul(
                                state["ps"],
                                lhsT=wq_sb[:, 2 * hp : 2 * hp + 2, ot * P : (ot + 1) * P],
                                rhs=hid_ts[hp],
                                start=(hp == 0),
                                stop=(hp == 7),
                                perf_mode=DR,
                            )
                            if hp == 7:
                                # evacuate on DVE: keeps ScalarE exclusively on
                                # Exp (ACT table switches cost ~2.7us each).
                                # out = (psum + bias*2^10) * descale
                                dst = (
                                    vT_sb[:, hl, sc * 512 : (sc + 1) * 512]
                                    if t == 2
                                    else qk_sbs[b][:, hl * 2 + t, sc * 512 : (sc + 1) * 512]
                                )
                                desc = float(
                                    (INV if t == 0 else np.float32(1.0))
                                    * np.float32(2.0**-WSCALE)
                                )
                                nc.vector.tensor_scalar(
                                    dst, state["ps"],
                                    bq_sb[:, ot : ot + 1], desc,
                                    mybir.AluOpType.add, mybir.AluOpType.mult,
                                )
                                if t == 2:
                                    # V^T chunk ready: PE-transpose its 4
                                    # k-tiles (psum slots borrowed from the
                                    # qkv pool), scale rows into fp8
                                    for kk in range(4):
                                        kt = sc * 4 + kk
                                        acol = (b * HPC + hl) * 16 + kt
                                        vt_ps = qps.tile(
                                            [P, P], BF16, tag="qkvps", bufs=2,
                                            name=f"vt_{b}_{sc}_{hl}_{kk}",
                                        )
                                        nc.tensor.transpose(
                                            vt_ps,
                                            vT_sb[:, hl, kt * P : (kt + 1) * P],
                                            eye_sb,
                                        )
                                        nc.vector.tensor_scalar_mul(
                                            v_sbs[b][:, hl, kt, :],
                                            vt_ps,
                                            ale_sb[:, acol : acol + 1],
                                        )
                                        nc.vector.tensor_scalar_mul(
                                            ow_sbs[b][:, hl, kt, :],
                                            ones_sb,
                                            ale_sb[:, 64 + acol : 64 + acol + 1],
                                        )

                        return emit

                    vT0 = vtp.tile([P, HPC, S], BF16, tag="vT", name="vT0")
                    for sc in range(4):
                        emit = qkv_sc(0, sc, vT0)
                        for j in range(48):
                            emit(j)
                    # mask loads on the idle gpsimd SWDGE queues, deferred so
                    # they don't steal head bandwidth from wq/hid
                    for i in range(4):
                        nc.gpsimd.dma_start(
                            mask_sb[:, i], mask01T[:, i * 8192 : (i + 1) * 8192]
                        )

                    # ---------- phase 2: attention(b0) + QKV(b1) ----------
                    with (
                        tc.tile_pool(name="att", bufs=3) as attp,
                        tc.tile_pool(name="aps", bufs=1, space="PSUM") as aps,
                    ):
                        vT1 = vtp.tile([P, HPC, S], BF16, tag="vT", name="vT1")
                        for qc in range(4):
                            for hl in range(HPC):
                                # 24 QKV(b1) matmuls woven into each block:
                                # 3 MMs per k-tile pair.
                                if hl == 0:
                                    emit = qkv_sc(1, qc, vT1)
                                base = 24 * hl

                                def extra(kp, emit=emit, base=base):
                                    for j in range(3):
                                        emit(base + kp * 3 + j)

                                attn_block(0, qc, hl, aps, attp, extra)
                            all_gather(0, qc)

                # ---------- phase 3: attention(b1) + dense(b0 + b1 early) --
                with (
                    tc.tile_pool(name="dw", bufs=1) as dwp,
                    tc.tile_pool(name="dctx", bufs=8) as dctxp,
                    tc.tile_pool(name="dps", bufs=2, space="PSUM") as dps,
                    tc.tile_pool(name="dout", bufs=3) as doutp,
                ):
                    wd_sb = dwp.tile([P, 16, DSH], F8)
                    nc.sync.dma_start(wd_sb, wdT.rearrange("(ht p) o -> p ht o", p=P))
                    # wd with h-tiles in [even heads | odd heads] order, for
                    # the tail chunk that consumes the per-head qc3 gathers
                    wd_sb_p = dwp.tile([P, 16, DSH], F8)
                    for t, ht in enumerate(list(range(0, 16, 2)) + list(range(1, 16, 2))):
                        nc.sync.dma_start(
                            wd_sb_p[:, t, :], wdT[ht * P : (ht + 1) * P, :]
                        )
                    rs_sb = dwp.tile([P, 2, B * S], F32)
                    nc.sync.dma_start(rs_sb, residT.rearrange("(ot p) s -> p ot s", p=P))

                    def dense_src(sc):
                        """cc_out chunk for output column chunk sc."""
                        return cc_out[sc // 4][sc % 4]

                    def dense_sc(sc):
                        """One 512-wide output column chunk: 2 o-tiles x 8
                        h-tile-pairs (fp8 DR); emit(j) for j in range(16)."""
                        src = dense_src(sc)
                        state = {}

                        def load(g):
                            # one 256KB DMA covers 2 h-tile pairs (4 h-tiles)
                            t = dctxp.tile([P, 4, 512], F8, tag="dctx", name="dctx_t")
                            nc.sync.dma_start(
                                t,
                                src[4 * g * P : (4 * g + 4) * P, :].rearrange(
                                    "(a p) q -> p a q", p=P
                                ),
                            )
                            state[f"c{g}"] = t

                        def emit(j):
                            hp, ot = divmod(j, 2)
                            if j == 0:
                                load(0)
                                load(1)
                            elif j in (4, 8):
                                load(j // 4 + 1)
                            state["ctx"] = state[f"c{hp // 2}"]
                            if hp == 0:
                                state[f"ps{ot}"] = dps.tile(
                                    [P, 512], F32, tag="dps", bufs=2,
                                    name=f"dps_{sc}_{ot}",
                                )
                            nc.tensor.matmul(
                                state[f"ps{ot}"],
                                lhsT=wd_sb[:, 2 * hp : 2 * hp + 2, ot * P : (ot + 1) * P],
                                rhs=state["ctx"][:, 2 * (hp % 2) : 2 * (hp % 2) + 2, :],
                                start=(hp == 0),
                                stop=(hp == 7),
                                perf_mode=DR,
                            )
                            if j == 15:
                                for o in range(2):
                                    o_t = doutp.tile([P, 512], F32, tag="o")
                                    nc.vector.tensor_add(
                                        o_t,
                                        state[f"ps{o}"],
                                        rs_sb[:, o, sc * 512 : (sc + 1) * 512],
                                    )
                                    nc.sync.dma_start(
                                        outT[o * P : (o + 1) * P, sc * 512 : (sc + 1) * 512],
                                        o_t,
                                    )

                        return emit

                    def dense_sc7():
                        """Tail chunk (b1 qc3 columns) against the per-head
                        gathers: h-tile pairs 0..3 = even heads (cc_out13[0]),
                        4..7 = odd heads (cc_out13[1]); wd_sb_p matches."""
                        state = {}

                        def load(g):
                            t = dctxp.tile([P, 4, 512], F8, tag="dctx", name="dctx_t")
                            src7 = cc_out13[g // 2]
                            r0 = (g % 2) * 4 * P
                            nc.sync.dma_start(
                                t,
                                src7[r0 : r0 + 4 * P, :].rearrange(
                                    "(a p) q -> p a q", p=P
                                ),
                            )
                            state[f"c{g}"] = t

                        def emit(j):
                            hp, ot = divmod(j, 2)
                            if j == 0:
                                load(0)
                                load(1)
                            elif j == 8:
                                load(2)
                                load(3)
                            if hp == 0:
                                state[f"ps{ot}"] = dps.tile(
                                    [P, 512], F32, tag="dps", bufs=2,
                                    name=f"dps_7_{ot}",
                                )
                            nc.tensor.matmul(
                                state[f"ps{ot}"],
                                lhsT=wd_sb_p[:, 2 * hp : 2 * hp + 2, ot * P : (ot + 1) * P],
                                rhs=state[f"c{hp // 2}"][:, 2 * (hp % 2) : 2 * (hp % 2) + 2, :],
                                start=(hp == 0),
                                stop=(hp == 7),
                                perf_mode=DR,
                            )
                            if j == 15:
                                for o in range(2):
                                    o_t = doutp.tile([P, 512], F32, tag="o")
                                    nc.vector.tensor_add(
                                        o_t,
                                        state[f"ps{o}"],
                                        rs_sb[:, o, 7 * 512 : 8 * 512],
                                    )
                                    nc.sync.dma_start(
                                        outT[o * P : (o + 1) * P, 7 * 512 : 8 * 512],
                                        o_t,
                                    )

                        return emit

                    with (
                        tc.tile_pool(name="att1", bufs=3) as attp,
                        tc.tile_pool(name="aps1", bufs=1, space="PSUM") as aps,
                    ):
                        # blocks 0..7 = (qc, hl); dense chunks sc0..sc6
                        # woven so each chunk's gather has landed before its
                        # block starts (b0 quarters land during phase 2; b1
                        # quarter qc lands ~1.5 blocks after block 2qc+1).
                        DENSE_AT = {0: [0], 1: [1], 2: [2], 4: [3], 5: [4], 6: [5], 7: [6]}
                        for qc in range(4):
                            for hl in range(HPC):
                                blk = qc * 2 + hl
                                if blk in DENSE_AT:
                                    emits = [dense_sc(s) for s in DENSE_AT[blk]]

                                    def extra(kp, emits=emits):
                                        for em in emits:
                                            for j in range(2):
                                                em(kp * 2 + j)
                                else:
                                    def extra(kp):
                                        pass
                                attn_block(1, qc, hl, aps, attp, extra)
                                if qc == 3:
                                    # per-head gather: hl0's half fires a
                                    # block early, hl1's is the (half-size)
                                    # tail gather
                                    all_gather13(hl)
                            if qc < 3:
                                all_gather(1, qc)

                    # ---------- phase 4: dense tail (last b1 column chunk) --
                    emit = dense_sc7()
                    for j in range(16):
                        emit(j)

    nc.compile()
    return nc


def _prep_in_maps(hidden_states, residual, alibi, attention_mask, w_qkv, b_qkv, w_dense, b_dense):
    f32 = np.float32
    wsc = f32(2.0**WSCALE)
    hs = np.asarray(hidden_states, f32).reshape(B * S, H)
    hidT = np.ascontiguousarray(hs.T).astype(NPF8)
    mask_keep = ~np.asarray(attention_mask).reshape(S, S)
    # [k, q] mask retiled to [p, qc, kt, q] so each (qc, k-pair) slice the
    # kernel reads is contiguous
    mkT = np.ascontiguousarray(mask_keep.T)          # [k, q]
    mask01T = np.ascontiguousarray(
        mkT.reshape(16, P, 4, 512).transpose(1, 2, 0, 3).reshape(P, 4 * 16 * 512)
    ).astype(NPBF16)
    ones_np = np.ones((P, P), f32).astype(NPBF16)
    al = np.asarray(alibi, f32).reshape(B, NH, S)
    resid = np.asarray(residual, f32).reshape(B * S, H)
    wq = np.asarray(w_qkv, f32)
    bq = np.asarray(b_qkv, f32)
    wd = np.asarray(w_dense, f32)
    bd = np.asarray(b_dense, f32)

    in_maps = []
    for r in range(NCORES):
        wshard = wq[r * OSH : (r + 1) * OSH] * wsc
        bshard = bq[r * OSH : (r + 1) * OSH] * wsc
        alcols_v = []
        alcols_o = []
        for b in range(B):
            for hl in range(HPC):
                e = np.exp(al[b, HPC * r + hl]).reshape(16, P).T
                alcols_v.append(e * f32(2.0**VSCALE))
                alcols_o.append(e * f32(2.0**OWSCALE))
        alibi8 = np.concatenate(alcols_o, axis=1).astype(NPF8)
        in_maps.append(
            {
                "hidT": hidT,
                "wqkvT": np.ascontiguousarray(wshard.T).astype(NPF8),
                "bqkv": np.ascontiguousarray(bshard.reshape(6, P).T),
                "mask01T": mask01T,
                "alibi_e": np.ascontiguousarray(
                    np.concatenate(alcols_v + alcols_o, axis=1)
                ),
                "alibi8": np.ascontiguousarray(alibi8),
                "wdT": np.ascontiguousarray(wd[r * DSH : (r + 1) * DSH].T * wsc).astype(NPF8),
                "residT": (
                    np.ascontiguousarray(resid[:, r * DSH : (r + 1) * DSH].T)
                    + bd[r * DSH : (r + 1) * DSH][:, None]
                ) / OUT_DESCALE,
                "ones": ones_np,
                "eye": np.eye(P, dtype=f32).astype(NPBF16),
            }
        )
    return in_maps


if os.environ.get("BASS_LDW_OPT"):
    _orig_run_command = bass_utils.run_command

    def _run_command_ldwopt(argv, **kwargs):
        argv = [
            "--enable-ldw-opt=true" if a == "--enable-ldw-opt=false" else a
            for a in argv
        ]
        return _orig_run_command(argv, **kwargs)

    bass_utils.run_command = _run_command_ldwopt


_NC_CACHE = {}


def run(inputs: dict, trace: bool = False):
    in_maps = _prep_in_maps(**inputs)
    if "nc" not in _NC_CACHE:
        _NC_CACHE["nc"] = build_nc()
    nc = _NC_CACHE["nc"]
    res = bass_utils.run_bass_kernel_spmd(
        nc, in_maps, core_ids=list(range(NCORES)), trace=trace
    )
    out = np.empty((B * S, H), np.float32)
    for r in range(NCORES):
        out[:, r * DSH : (r + 1) * DSH] = res.results[r]["outT"].T * OUT_DESCALE
    return out.reshape(B, S, H), res


def kernel(**inputs) -> np.ndarray:
    out, _ = run(inputs, trace=False)
    return out


# revision 19
# speedup vs baseline: 1.0095x; 1.0095x over previous
# Bloom parallel attention block on 8 trn2 NeuronCores, tensor-parallel over
# heads (2 heads per core).  Feature-major layouts throughout.
#
# fp8 (e4m3, max 240) + DoubleRow variant: QKV, ctx, softmax-denominator and
# dense matmuls run as K=256 fp8 DoubleRow matmuls (2x PE throughput); the
# score matmuls stay bf16 (K=128, no DR gain).  Power-of-2 scales keep fp8
# operands out of the subnormal range:
#   wqkv, wdense scaled 2^10 on host; descale folded into the QKV psum
#   evacuation (tensor_scalar mult) and the host-side output descale.
#   v' = v * exp(alibi) * 2^4,  ones' = exp(alibi) * 2^-1  =>  ctx = 2^5 ctx.
#   probs = exp(scores - 5.5) via ACT bias (global shift cancels in the
#   softmax normalization; keeps exp <= 240 for fp8e4).
#   dense out = 2^(5+10) * true; residual pre-scaled 2^15 on host, output
#   descaled 2^-15 on host.
# ctx is gathered in fp8 (halves collective bytes vs bf16).
#
# Per core r (heads 2r, 2r+1):
#   QKV DR matmul -> Q^T/K^T [d, s] bf16 and V^T [d, s] bf16 per batch.
#   V^T is PE-transposed to V [s, d] and scaled by exp(alibi[k])*2^4 into
#   fp8; ones' tiles = exp(alibi[k])*2^-1 broadcast columns (fp8).
#   attention (per b, head hl, 512-wide q-chunk qc), scores transposed [k, q]:
#     scores^T = K^T_tile.T @ Q^T       x2 k-tiles  (PE bf16, fp32 psum)
#     exp(scores - 5.5) over both k-tiles (ACT, bf16 out)
#     * mask01^T                        (DVE, 0/1 mask, fp8e4 out)
#     ctx^T += V'_pair.T @ probs        (PE fp8 DoubleRow, 4 k-tile pairs..)
#     sum   += ones'_pair.T @ probs     (PE fp8 DoubleRow, accum 8 pairs)
#     ctx^T *= 1/sum -> fp8 -> DMA to cc chunk
#   Pipelining: QKV(b1) matmuls are interleaved into attention(b0) k-loops,
#   dense matmuls into attention(b1) k-loops, so the PE never idles.  ctx
#   is AllGathered in 6 fp8 column chunks (2 for b0, 4 for b1).
#   dense: out^T[o_local, s] = wdT_pair.T @ ctx^T_full (fp8 DR) + resid'
#     (column-parallel => no all-reduce; host concatenates output slices)
import os
import sys

import numpy as np

if "/opt/trn_rl_repo" not in sys.path:
    sys.path.insert(0, "/opt/trn_rl_repo")

import ml_dtypes

import concourse.bass as bass
import concourse.mybir as mybir
import concourse.tile as tile
from concourse import bacc, bass_utils

B, S, H, NH = 2, 2048, 2048, 16
HD = H // NH            # 128
NCORES = 8
HPC = NH // NCORES      # heads per core = 2
OSH = 3 * H // NCORES   # qkv output rows per core = 768
DSH = H // NCORES       # dense output cols per core = 256
P = 128
F32 = mybir.dt.float32
BF16 = mybir.dt.bfloat16
F8 = mybir.dt.float8e4
AF = mybir.ActivationFunctionType
DR = mybir.MatmulPerfMode.DoubleRow
NPBF16 = ml_dtypes.bfloat16
NPF8 = ml_dtypes.float8_e4m3

WSCALE = 10          # wqkv/wdense host scale 2^10
VSCALE = 4           # v' = v*e*2^4
OWSCALE = -1         # ones' = e*2^-1  => ctxn = 2^5 ctx
ESHIFT = 5.5         # probs = exp(s - 5.5); cancels in normalization
OUT_DESCALE = np.float32(2.0 ** -(5 + WSCALE))   # host output descale
INV = np.float32(1.0 / np.sqrt(HD))


def build_nc():
    nc = bacc.Bacc(
        "TRN2",
        target_bir_lowering=False,
        debug=False,
        num_devices=NCORES,
    )

    hidT = nc.dram_tensor("hidT", [H, B * S], F8, kind="ExternalInput").ap()
    wqkvT = nc.dram_tensor("wqkvT", [H, OSH], F8, kind="ExternalInput").ap()
    bqkv = nc.dram_tensor("bqkv", [P, 6], F32, kind="ExternalInput").ap()
    mask01T = nc.dram_tensor("mask01T", [P, 4 * 16 * 512], BF16, kind="ExternalInput").ap()
    alibi_e = nc.dram_tensor("alibi_e", [P, 2 * 2 * HPC * 16], F32, kind="ExternalInput").ap()
    ow8 = nc.dram_tensor("ow8", [P, 2 * HPC * 16 * P], F8, kind="ExternalInput").ap()
    wdT = nc.dram_tensor("wdT", [H, DSH], F8, kind="ExternalInput").ap()
    residT = nc.dram_tensor("residT", [DSH, B * S], F32, kind="ExternalInput").ap()
    ones = nc.dram_tensor("ones", [P, P], BF16, kind="ExternalInput").ap()
    eye = nc.dram_tensor("eye", [P, P], BF16, kind="ExternalInput").ap()
    outT = nc.dram_tensor("outT", [DSH, B * S], F32, kind="ExternalOutput").ap()

    with tile.TileContext(nc) as tc:
        ccg = [list(range(NCORES))]
        with (
            tc.tile_pool(name="const", bufs=1) as constp,
            tc.tile_pool(name="dram", bufs=1, space="DRAM") as dramp,
        ):
            bq_sb = constp.tile([P, 6], F32)
            nc.gpsimd.dma_start(bq_sb, bqkv)
            ale_sb = constp.tile([P, 2 * 2 * HPC * 16], F32)
            nc.gpsimd.dma_start(ale_sb, alibi_e)

            ones_sb = constp.tile(
                [P, P], BF16,
                name="ones_sb_ldw" if os.environ.get("BASS_LDW_OPT") else "ones_sb",
            )
            nc.gpsimd.dma_start(ones_sb, ones)
            eye_sb = constp.tile([P, P], BF16)
            nc.gpsimd.dma_start(eye_sb, eye)
            esh_sb = constp.tile([P, 1], F32)
            nc.gpsimd.memset(esh_sb, -ESHIFT)

            # ctx gather chunks (fp8): both batches in 4 column quarters,
            # each fired as soon as its q-chunk's two heads finish
            cc_spec = [(4, S // 4), (4, S // 4)]
            cc_in = [
                [
                    dramp.tile([HPC * HD, w], F8, name=f"cc_in{b}{i}")
                    for i in range(n)
                ]
                for b, (n, w) in enumerate(cc_spec)
            ]
            cc_out = [
                [
                    dramp.tile([H, w], F8, addr_space="Shared", name=f"cc_out{b}{i}")
                    for i in range(n)
                ]
                for b, (n, w) in enumerate(cc_spec)
            ]
            # b1 qc3 is gathered per-head so the first half fires one block
            # earlier and the tail gather is half-size.  Gathered row order
            # becomes [all even heads | all odd heads]; the tail dense chunk
            # uses a correspondingly permuted copy of wd.
            cc_in13 = [dramp.tile([P, 512], F8, name=f"cc_in13h{h}") for h in range(2)]
            cc_out13 = [
                dramp.tile([NCORES * P, 512], F8, addr_space="Shared", name=f"cc_out13h{h}")
                for h in range(2)
            ]

            def dma_ctx(b, qc, hl, ctxn_t):
                if b == 1 and qc == 3:
                    nc.sync.dma_start(cc_in13[hl], ctxn_t)
                    return
                n, w = cc_spec[b]
                chunk, qq = divmod(qc, 4 // n)
                nc.sync.dma_start(
                    cc_in[b][chunk][hl * P : (hl + 1) * P, qq * 512 : (qq + 1) * 512],
                    ctxn_t,
                )

            def all_gather(b, chunk):
                nc.gpsimd.collective_compute(
                    "AllGather", mybir.AluOpType.bypass, replica_groups=ccg,
                    ins=[cc_in[b][chunk].opt()], outs=[cc_out[b][chunk].opt()],
                )

            def all_gather13(h):
                nc.gpsimd.collective_compute(
                    "AllGather", mybir.AluOpType.bypass, replica_groups=ccg,
                    ins=[cc_in13[h].opt()], outs=[cc_out13[h].opt()],
                )

            with (
                tc.tile_pool(name="mask", bufs=1) as maskp,
                tc.tile_pool(name="qk1", bufs=1) as qk1p,
                tc.tile_pool(name="vt", bufs=1) as vtp,
                tc.tile_pool(name="v1", bufs=1) as v1p,
                tc.tile_pool(name="ow1", bufs=1) as ow1p,
            ):
                mask_sb = maskp.tile([P, 4, 16, 512], BF16)
                qk_sbs = [None, qk1p.tile([P, 2 * HPC, S], BF16, name="qksb1")]
                v_sbs = [None, v1p.tile([P, HPC, 16, P], F8, name="vsb1")]
                # ones' tiles (exp(alibi)*2^-1 broadcast columns) are
                # precomputed on host and DMAed, not built on the DVE
                ow_sbs = [None, ow1p.tile([P, HPC, 16, P], F8, name="owsb1")]
                nc.gpsimd.dma_start(ow_sbs[1], ow8[:, HPC * 16 * P :])

                def attn_block(b, qc, hl, aps, attp, extra_mm):
                    """Attention for (b, head hl, q-chunk qc), k-tiles in
                    pairs; extra_mm(kp) emits extra matmuls per pair to keep
                    the PE busy while ACT/DVE run."""
                    qk = qk_sbs[b]
                    ctx_ps = aps.tile([P, 512], F32, tag="ctx", bufs=1)
                    sum_ps = aps.tile([P, 512], F32, tag="sum", bufs=1)
                    for kp in range(8):
                        kt0 = 2 * kp
                        s_ps = aps.tile([P, 1024], F32, tag="sco", bufs=2)
                        for u in range(2):
                            nc.tensor.matmul(
                                s_ps[:, u * 512 : (u + 1) * 512],
                                lhsT=qk[:, hl * 2 + 1, (kt0 + u) * P : (kt0 + u + 1) * P],
                                rhs=qk[:, hl * 2, qc * 512 : (qc + 1) * 512],
                                start=True,
                                stop=True,
                            )
                        exp_t = attp.tile([P, 1024], BF16, tag="exp")
                        nc.scalar.activation(exp_t, s_ps, AF.Exp, bias=esh_sb)
                        prob_t = attp.tile([P, 2, 512], F8, tag="prob")
                        nc.vector.tensor_mul(
                            prob_t,
                            exp_t.rearrange("p (u q) -> p u q", u=2),
                            mask_sb[:, qc, kt0 : kt0 + 2, :],
                        )
                        # weave BEFORE ctx/sum: the PE queue is FIFO, so the
                        # independent woven matmuls must sit ahead of the
                        # prob-dependent ones to fill the exp/prob latency
                        extra_mm(kp)
                        nc.tensor.matmul(
                            ctx_ps,
                            lhsT=v_sbs[b][:, hl, kt0 : kt0 + 2, :],
                            rhs=prob_t,
                            start=(kp == 0),
                            stop=(kp == 7),
                            perf_mode=DR,
                        )
                        nc.tensor.matmul(
                            sum_ps,
                            lhsT=ow_sbs[b][:, hl, kt0 : kt0 + 2, :],
                            rhs=prob_t,
                            start=(kp == 0),
                            stop=(kp == 7),
                            perf_mode=DR,
                        )
                    rec_t = attp.tile([P, 512], F32, tag="rec", bufs=2)
                    nc.vector.reciprocal_approx_fast(rec_t, sum_ps)
                    ctxn_t = attp.tile([P, 512], F8, tag="ctxn", bufs=2)
                    nc.vector.tensor_mul(ctxn_t, ctx_ps, rec_t)
                    dma_ctx(b, qc, hl, ctxn_t)

                # ---------- phase 1: QKV(b0), standalone ----------
                with (
                    tc.tile_pool(name="qk0", bufs=1) as qk0p,
                    tc.tile_pool(name="v0", bufs=1) as v0p,
                    tc.tile_pool(name="ow0", bufs=1) as ow0p,
                    tc.tile_pool(name="wq", bufs=1) as wqp,
                    tc.tile_pool(name="hid", bufs=16) as hidp,
                    tc.tile_pool(name="qps", bufs=3, space="PSUM") as qps,
                ):
                    qk_sbs[0] = qk0p.tile([P, 2 * HPC, S], BF16, name="qksb0")
                    v_sbs[0] = v0p.tile([P, HPC, 16, P], F8, name="vsb0")
                    ow_sbs[0] = ow0p.tile([P, HPC, 16, P], F8, name="owsb0")
                    nc.gpsimd.dma_start(ow_sbs[0], ow8[:, : HPC * 16 * P])
                    wq_sb = wqp.tile([P, 16, OSH], F8)

                    def qkv_sc(b, sc, vT_sb):
                        """QKV for one 512-wide s-chunk: 6 o-tiles x 8
                        h-tile-pairs (fp8 DoubleRow); call emit(j) for j in
                        range(48).  V^T o-tiles are PE-transposed to V [k, d]
                        and scaled by exp(alibi[k])*2^4 into fp8; ones' tiles
                        built alongside at 2^-1."""
                        hid_ts = []
                        for hp in range(8):
                            if b == 0 and sc == 0:
                                nc.gpsimd.dma_start(
                                    wq_sb[:, 2 * hp : 2 * hp + 2, :],
                                    wqkvT[2 * hp * P : (2 * hp + 2) * P, :].rearrange(
                                        "(two p) o -> p two o", p=P
                                    ),
                                )
                            hid_t = hidp.tile([P, 2, 512], F8, tag="hid")
                            nc.sync.dma_start(
                                hid_t,
                                hidT[
                                    2 * hp * P : (2 * hp + 2) * P,
                                    b * S + sc * 512 : b * S + (sc + 1) * 512,
                                ].rearrange("(two p) s -> p two s", p=P),
                            )
                            hid_ts.append(hid_t)
                        state = {"ps": None}

                        def emit(j):
                            ot, hp = divmod(j, 8)
                            hl, t = divmod(ot, 3)
                            if hp == 0:
                                state["ps"] = qps.tile(
                                    [P, 512], F32, tag="qkvps", bufs=2,
                                    name=f"qps_{b}_{sc}_{ot}",
                                )
                            nc.tensor.matmul(
                                state["ps"],
                                lhsT=wq_sb[:, 2 * hp : 2 * hp + 2, ot * P : (ot + 1) * P],
                                rhs=hid_ts[hp],
                                start=(hp == 0),
                                stop=(hp == 7),
                                perf_mode=DR,
                            )
                            if hp == 7:
                                # evacuate on DVE: keeps ScalarE exclusively on
                                # Exp (ACT table switches cost ~2.7us each).
                                # out = (psum + bias*2^10) * descale
                                dst = (
                                    vT_sb[:, hl, sc * 512 : (sc + 1) * 512]
                                    if t == 2
                                    else qk_sbs[b][:, hl * 2 + t, sc * 512 : (sc + 1) * 512]
                                )
                                desc = float(
                                    (INV if t == 0 else np.float32(1.0))
                                    * np.float32(2.0**-WSCALE)
                                )
                                nc.vector.tensor_scalar(
                                    dst, state["ps"],
                                    bq_sb[:, ot : ot + 1], desc,
                                    mybir.AluOpType.add, mybir.AluOpType.mult,
                                )
                                if t == 2:
                                    # V^T chunk ready: PE-transpose its 4
                                    # k-tiles (psum slots borrowed from the
                                    # qkv pool), scale rows into fp8
                                    for kk in range(4):
                                        kt = sc * 4 + kk
                                        acol = (b * HPC + hl) * 16 + kt
                                        vt_ps = qps.tile(
                                            [P, P], BF16, tag="qkvps", bufs=2,
                                            name=f"vt_{b}_{sc}_{hl}_{kk}",
                                        )
                                        nc.tensor.transpose(
                                            vt_ps,
                                            vT_sb[:, hl, kt * P : (kt + 1) * P],
                                            eye_sb,
                                        )
                                        nc.vector.tensor_scalar_mul(
                                            v_sbs[b][:, hl, kt, :],
                                            vt_ps,
                                            ale_sb[:, acol : acol + 1],
                                        )

                        return emit

                    vT0 = vtp.tile([P, HPC, S], BF16, tag="vT", name="vT0")
                    for sc in range(4):
                        emit = qkv_sc(0, sc, vT0)
                        for j in range(48):
                            emit(j)
                    # mask loads on the idle gpsimd SWDGE queues, deferred so
                    # they don't steal head bandwidth from wq/hid
                    for i in range(4):
                        nc.gpsimd.dma_start(
                            mask_sb[:, i], mask01T[:, i * 8192 : (i + 1) * 8192]
                        )

                    # ---------- phase 2: attention(b0) + QKV(b1) ----------
                    with (
                        tc.tile_pool(name="att", bufs=3) as attp,
                        tc.tile_pool(name="aps", bufs=1, space="PSUM") as aps,
                    ):
                        vT1 = vtp.tile([P, HPC, S], BF16, tag="vT", name="vT1")
                        for qc in range(4):
                            for hl in range(HPC):
                                # 24 QKV(b1) matmuls woven into each block:
                                # 3 MMs per k-tile pair.
                                if hl == 0:
                                    emit = qkv_sc(1, qc, vT1)
                                base = 24 * hl

                                def extra(kp, emit=emit, base=base):
                                    for j in range(3):
                                        emit(base + kp * 3 + j)

                                attn_block(0, qc, hl, aps, attp, extra)
                            all_gather(0, qc)

                # ---------- phase 3: attention(b1) + dense(b0 + b1 early) --
                with (
                    tc.tile_pool(name="dw", bufs=1) as dwp,
                    tc.tile_pool(name="dctx", bufs=8) as dctxp,
                    tc.tile_pool(name="dps", bufs=2, space="PSUM") as dps,
                    tc.tile_pool(name="dout", bufs=3) as doutp,
                ):
                    wd_sb = dwp.tile([P, 16, DSH], F8)
                    nc.sync.dma_start(wd_sb, wdT.rearrange("(ht p) o -> p ht o", p=P))
                    # wd with h-tiles in [even heads | odd heads] order, for
                    # the tail chunk that consumes the per-head qc3 gathers
                    wd_sb_p = dwp.tile([P, 16, DSH], F8)
                    for t, ht in enumerate(list(range(0, 16, 2)) + list(range(1, 16, 2))):
                        nc.sync.dma_start(
                            wd_sb_p[:, t, :], wdT[ht * P : (ht + 1) * P, :]
                        )
                    rs_sb = dwp.tile([P, 2, B * S], F32)
                    nc.sync.dma_start(rs_sb, residT.rearrange("(ot p) s -> p ot s", p=P))

                    def dense_src(sc):
                        """cc_out chunk for output column chunk sc."""
                        return cc_out[sc // 4][sc % 4]

                    def dense_sc(sc):
                        """One 512-wide output column chunk: 2 o-tiles x 8
                        h-tile-pairs (fp8 DR); emit(j) for j in range(16)."""
                        src = dense_src(sc)
                        state = {}

                        def load(g):
                            # one 256KB DMA covers 2 h-tile pairs (4 h-tiles)
                            t = dctxp.tile([P, 4, 512], F8, tag="dctx", name="dctx_t")
                            nc.sync.dma_start(
                                t,
                                src[4 * g * P : (4 * g + 4) * P, :].rearrange(
                                    "(a p) q -> p a q", p=P
                                ),
                            )
                            state[f"c{g}"] = t

                        def emit(j):
                            hp, ot = divmod(j, 2)
                            if j == 0:
                                load(0)
                                load(1)
                            elif j in (4, 8):
                                load(j // 4 + 1)
                            state["ctx"] = state[f"c{hp // 2}"]
                            if hp == 0:
                                state[f"ps{ot}"] = dps.tile(
                                    [P, 512], F32, tag="dps", bufs=2,
                                    name=f"dps_{sc}_{ot}",
                                )
                            nc.tensor.matmul(
                                state[f"ps{ot}"],
                                lhsT=wd_sb[:, 2 * hp : 2 * hp + 2, ot * P : (ot + 1) * P],
                                rhs=state["ctx"][:, 2 * (hp % 2) : 2 * (hp % 2) + 2, :],
                                start=(hp == 0),
                                stop=(hp == 7),
                                perf_mode=DR,
                            )
                            if j == 15:
                                for o in range(2):
                                    o_t = doutp.tile([P, 512], F32, tag="o")
                                    nc.vector.tensor_add(
                                        o_t,
                                        state[f"ps{o}"],
                                        rs_sb[:, o, sc * 512 : (sc + 1) * 512],
                                    )
                                    nc.sync.dma_start(
                                        outT[o * P : (o + 1) * P, sc * 512 : (sc + 1) * 512],
                                        o_t,
                                    )

                        return emit

                    def dense_sc7():
                        """Tail chunk (b1 qc3 columns) against the per-head
                        gathers: h-tile pairs 0..3 = even heads (cc_out13[0]),
                        4..7 = odd heads (cc_out13[1]); wd_sb_p matches."""
                        state = {}

                        def load(g):
                            t = dctxp.tile([P, 4, 512], F8, tag="dctx", name="dctx_t")
                            src7 = cc_out13[g // 2]
                            r0 = (g % 2) * 4 * P
                            nc.sync.dma_start(
                                t,
                                src7[r0 : r0 + 4 * P, :].rearrange(
                                    "(a p) q -> p a q", p=P
                                ),
                            )
                            state[f"c{g}"] = t

                        def emit(j):
                            hp, ot = divmod(j, 2)
                            if j == 0:
                                load(0)
                                load(1)
                            elif j == 8:
                                load(2)
                                load(3)
                            if hp == 0:
                                state[f"ps{ot}"] = dps.tile(
                                    [P, 512], F32, tag="dps", bufs=2,
                                    name=f"dps_7_{ot}",
                                )
                            nc.tensor.matmul(
                                state[f"ps{ot}"],
                                lhsT=wd_sb_p[:, 2 * hp : 2 * hp + 2, ot * P : (ot + 1) * P],
                                rhs=state[f"c{hp // 2}"][:, 2 * (hp % 2) : 2 * (hp % 2) + 2, :],
                                start=(hp == 0),
                                stop=(hp == 7),
                                perf_mode=DR,
                            )
                            if j == 15:
                                for o in range(2):
                                    o_t = doutp.tile([P, 512], F32, tag="o")
                                    nc.vector.tensor_add(
                                        o_t,
                                        state[f"ps{o}"],
                                        rs_sb[:, o, 7 * 512 : 8 * 512],
                                    )
                                    nc.sync.dma_start(
                                        outT[o * P : (o + 1) * P, 7 * 512 : 8 * 512],
                                        o_t,
                                    )

                        return emit

                    with (
                        tc.tile_pool(name="att1", bufs=3) as attp,
                        tc.tile_pool(name="aps1", bufs=1, space="PSUM") as aps,
                    ):
                        # blocks 0..7 = (qc, hl); dense chunks sc0..sc6
                        # woven so each chunk's gather has landed before its
                        # block starts (b0 quarters land during phase 2; b1
                        # quarter qc lands ~1.5 blocks after block 2qc+1).
                        DENSE_AT = {0: [0], 1: [1], 2: [2], 4: [3], 5: [4], 6: [5], 7: [6]}
                        for qc in range(4):
                            for hl in range(HPC):
                                blk = qc * 2 + hl
                                if blk in DENSE_AT:
                                    emits = [dense_sc(s) for s in DENSE_AT[blk]]

                                    def extra(kp, emits=emits):
                                        for em in emits:
                                            for j in range(2):
                                                em(kp * 2 + j)
                                else:
                                    def extra(kp):
                                        pass
                                attn_block(1, qc, hl, aps, attp, extra)
                                if qc == 3:
                                    # per-head gather: hl0's half fires a
                                    # block early, hl1's is the (half-size)
                                    # tail gather
                                    all_gather13(hl)
                            if qc < 3:
                                all_gather(1, qc)

                    # ---------- phase 4: dense tail (last b1 column chunk) --
                    emit = dense_sc7()
                    for j in range(16):
                        emit(j)

    nc.compile()
    return nc


def _prep_in_maps(hidden_states, residual, alibi, attention_mask, w_qkv, b_qkv, w_dense, b_dense):
    f32 = np.float32
    wsc = f32(2.0**WSCALE)
    hs = np.asarray(hidden_states, f32).reshape(B * S, H)
    hidT = np.ascontiguousarray(hs.T).astype(NPF8)
    mask_keep = ~np.asarray(attention_mask).reshape(S, S)
    # [k, q] mask retiled to [p, qc, kt, q] so each (qc, k-pair) slice the
    # kernel reads is contiguous
    mkT = np.ascontiguousarray(mask_keep.T)          # [k, q]
    mask01T = np.ascontiguousarray(
        mkT.reshape(16, P, 4, 512).transpose(1, 2, 0, 3).reshape(P, 4 * 16 * 512)
    ).astype(NPBF16)
    ones_np = np.ones((P, P), f32).astype(NPBF16)
    al = np.asarray(alibi, f32).reshape(B, NH, S)
    resid = np.asarray(residual, f32).reshape(B * S, H)
    wq = np.asarray(w_qkv, f32)
    bq = np.asarray(b_qkv, f32)
    wd = np.asarray(w_dense, f32)
    bd = np.asarray(b_dense, f32)

    in_maps = []
    for r in range(NCORES):
        wshard = wq[r * OSH : (r + 1) * OSH] * wsc
        bshard = bq[r * OSH : (r + 1) * OSH] * wsc
        alcols_v = []
        alcols_o = []
        for b in range(B):
            for hl in range(HPC):
                e = np.exp(al[b, HPC * r + hl]).reshape(16, P).T
                alcols_v.append(e * f32(2.0**VSCALE))
                alcols_o.append(e * f32(2.0**OWSCALE))
        # expanded ones' tiles: [p, (b, hl, kt, m)] = e[b,hl][kt*128+p]*2^-1
        ow8 = np.ascontiguousarray(
            np.stack([c.T for c in alcols_o], axis=0)       # [bhl, kt, p]
            .transpose(2, 0, 1)                              # [p, bhl, kt]
            .reshape(P, 2 * HPC * 16)[:, :, None]
            .repeat(P, axis=2)
            .reshape(P, 2 * HPC * 16 * P)
        ).astype(NPF8)
        in_maps.append(
            {
                "hidT": hidT,
                "wqkvT": np.ascontiguousarray(wshard.T).astype(NPF8),
                "bqkv": np.ascontiguousarray(bshard.reshape(6, P).T),
                "mask01T": mask01T,
                "alibi_e": np.ascontiguousarray(
                    np.concatenate(alcols_v + alcols_o, axis=1)
                ),
                "ow8": ow8,
                "wdT": np.ascontiguousarray(wd[r * DSH : (r + 1) * DSH].T * wsc).astype(NPF8),
                "residT": (
                    np.ascontiguousarray(resid[:, r * DSH : (r + 1) * DSH].T)
                    + bd[r * DSH : (r + 1) * DSH][:, None]
                ) / OUT_DESCALE,
                "ones": ones_np,
                "eye": np.eye(P, dtype=f32).astype(NPBF16),
            }
        )
    return in_maps


if os.environ.get("BASS_LDW_OPT"):
    _orig_run_command = bass_utils.run_command

    def _run_command_ldwopt(argv, **kwargs):
        argv = [
            "--enable-ldw-opt=true" if a == "--enable-ldw-opt=false" else a
            for a in argv
        ]
        return _orig_run_command(argv, **kwargs)

    bass_utils.run_command = _run_command_ldwopt


_NC_CACHE = {}


def run(inputs: dict, trace: bool = False):
    in_maps = _prep_in_maps(**inputs)
    if "nc" not in _NC_CACHE:
        _NC_CACHE["nc"] = build_nc()
    nc = _NC_CACHE["nc"]
    res = bass_utils.run_bass_kernel_spmd(
        nc, in_maps, core_ids=list(range(NCORES)), trace=trace
    )
    out = np.empty((B * S, H), np.float32)
    for r in range(NCORES):
        out[:, r * DSH : (r + 1) * DSH] = res.results[r]["outT"].T * OUT_DESCALE
    return out.reshape(B, S, H), res


def kernel(**inputs) -> np.ndarray:
    out, _ = run(inputs, trace=False)
    return out


# revision 20
# speedup vs baseline: 1.0642x; 1.0541x over previous
# Bloom parallel attention block on 8 trn2 NeuronCores, tensor-parallel over
# heads (2 heads per core).  Feature-major layouts throughout.
#
# fp8 (e4m3, max 240) + DoubleRow variant: QKV, ctx, softmax-denominator and
# dense matmuls run as K=256 fp8 DoubleRow matmuls (2x PE throughput); the
# score matmuls stay bf16 (K=128, no DR gain).  Power-of-2 scales keep fp8
# operands out of the subnormal range:
#   wqkv, wdense scaled 2^10 on host; descale folded into the QKV psum
#   evacuation (tensor_scalar mult) and the host-side output descale.
#   v' = v * exp(alibi) * 2^4,  ones' = exp(alibi) * 2^-1  =>  ctx = 2^5 ctx.
#   probs = exp(scores - 5.5) via ACT bias (global shift cancels in the
#   softmax normalization; keeps exp <= 240 for fp8e4).
#   dense out = 2^(5+10) * true; residual pre-scaled 2^15 on host, output
#   descaled 2^-15 on host.
# ctx is gathered in fp8 (halves collective bytes vs bf16).
#
# Per core r (heads 2r, 2r+1):
#   QKV DR matmul -> Q^T/K^T [d, s] bf16 and V^T [d, s] bf16 per batch.
#   V^T is PE-transposed to V [s, d] and scaled by exp(alibi[k])*2^4 into
#   fp8; ones' tiles = exp(alibi[k])*2^-1 broadcast columns (fp8).
#   attention (per b, head hl, 512-wide q-chunk qc), scores transposed [k, q]:
#     scores^T = K^T_tile.T @ Q^T       x2 k-tiles  (PE bf16, fp32 psum)
#     exp(scores - 5.5) over both k-tiles (ACT, bf16 out)
#     * mask01^T                        (DVE, 0/1 mask, fp8e4 out)
#     ctx^T += V'_pair.T @ probs        (PE fp8 DoubleRow, 4 k-tile pairs..)
#     sum   += ones'_pair.T @ probs     (PE fp8 DoubleRow, accum 8 pairs)
#     ctx^T *= 1/sum -> fp8 -> DMA to cc chunk
#   Pipelining: QKV(b1) matmuls are interleaved into attention(b0) k-loops,
#   dense matmuls into attention(b1) k-loops, so the PE never idles.  ctx
#   is AllGathered in 6 fp8 column chunks (2 for b0, 4 for b1).
#   dense: out^T[o_local, s] = wdT_pair.T @ ctx^T_full (fp8 DR) + resid'
#     (column-parallel => no all-reduce; host concatenates output slices)
import os
import sys

import numpy as np

if "/opt/trn_rl_repo" not in sys.path:
    sys.path.insert(0, "/opt/trn_rl_repo")

import ml_dtypes

import concourse.bass as bass
import concourse.mybir as mybir
import concourse.tile as tile
from concourse import bacc, bass_utils

B, S, H, NH = 2, 2048, 2048, 16
HD = H // NH            # 128
NCORES = 8
HPC = NH // NCORES      # heads per core = 2
OSH = 3 * H // NCORES   # qkv output rows per core = 768
DSH = H // NCORES       # dense output cols per core = 256
P = 128
F32 = mybir.dt.float32
BF16 = mybir.dt.bfloat16
F8 = mybir.dt.float8e4
AF = mybir.ActivationFunctionType
DR = mybir.MatmulPerfMode.DoubleRow
NPBF16 = ml_dtypes.bfloat16
NPF8 = ml_dtypes.float8_e4m3

WSCALE = 10          # wqkv/wdense host scale 2^10
VSCALE = 4           # v' = v*e*2^4
OWSCALE = -1         # ones' = e*2^-1  => ctxn = 2^5 ctx
ESHIFT = 5.5         # probs = exp(s - 5.5); cancels in normalization
OUT_DESCALE = np.float32(2.0 ** -(5 + WSCALE))   # host output descale
INV = np.float32(1.0 / np.sqrt(HD))


def build_nc():
    nc = bacc.Bacc(
        "TRN2",
        target_bir_lowering=False,
        debug=False,
        num_devices=NCORES,
    )

    hidT = nc.dram_tensor("hidT", [H, B * S], F8, kind="ExternalInput").ap()
    wqkvT = nc.dram_tensor("wqkvT", [H, OSH], F8, kind="ExternalInput").ap()
    bqkv = nc.dram_tensor("bqkv", [P, 6], F32, kind="ExternalInput").ap()
    mask01T = nc.dram_tensor("mask01T", [S, S], BF16, kind="ExternalInput").ap()
    alibi_e = nc.dram_tensor("alibi_e", [P, 2 * 2 * HPC * 16], F32, kind="ExternalInput").ap()
    wdT = nc.dram_tensor("wdT", [H, DSH], F8, kind="ExternalInput").ap()
    residT = nc.dram_tensor("residT", [DSH, B * S], F32, kind="ExternalInput").ap()
    ones = nc.dram_tensor("ones", [P, P], BF16, kind="ExternalInput").ap()
    eye = nc.dram_tensor("eye", [P, P], BF16, kind="ExternalInput").ap()
    outT = nc.dram_tensor("outT", [DSH, B * S], F32, kind="ExternalOutput").ap()

    with tile.TileContext(nc) as tc:
        ccg = [list(range(NCORES))]
        with (
            tc.tile_pool(name="const", bufs=1) as constp,
            tc.tile_pool(name="dram", bufs=1, space="DRAM") as dramp,
        ):
            bq_sb = constp.tile([P, 6], F32)
            nc.gpsimd.dma_start(bq_sb, bqkv)
            ale_sb = constp.tile([P, 2 * 2 * HPC * 16], F32)
            nc.gpsimd.dma_start(ale_sb, alibi_e)

            ones_sb = constp.tile(
                [P, P], BF16,
                name="ones_sb_ldw" if os.environ.get("BASS_LDW_OPT") else "ones_sb",
            )
            nc.gpsimd.dma_start(ones_sb, ones)
            eye_sb = constp.tile([P, P], BF16)
            nc.gpsimd.dma_start(eye_sb, eye)
            esh_sb = constp.tile([P, 1], F32)
            nc.gpsimd.memset(esh_sb, -ESHIFT)

            # ctx gather chunks (fp8): both batches in 4 column quarters,
            # each fired as soon as its q-chunk's two heads finish
            cc_spec = [(4, S // 4), (4, S // 4)]
            cc_in = [
                [
                    dramp.tile([HPC * HD, w], F8, name=f"cc_in{b}{i}")
                    for i in range(n)
                ]
                for b, (n, w) in enumerate(cc_spec)
            ]
            cc_out = [
                [
                    dramp.tile([H, w], F8, addr_space="Shared", name=f"cc_out{b}{i}")
                    for i in range(n)
                ]
                for b, (n, w) in enumerate(cc_spec)
            ]
            # b1 qc3 is gathered per-head so the first half fires one block
            # earlier and the tail gather is half-size.  Gathered row order
            # becomes [all even heads | all odd heads]; the tail dense chunk
            # uses a correspondingly permuted copy of wd.
            cc_in13 = [dramp.tile([P, 512], F8, name=f"cc_in13h{h}") for h in range(2)]
            cc_out13 = [
                dramp.tile([NCORES * P, 512], F8, addr_space="Shared", name=f"cc_out13h{h}")
                for h in range(2)
            ]

            def dma_ctx(b, qc, hl, ctxn_t):
                if b == 1 and qc == 3:
                    nc.sync.dma_start(cc_in13[hl], ctxn_t)
                    return
                n, w = cc_spec[b]
                chunk, qq = divmod(qc, 4 // n)
                nc.sync.dma_start(
                    cc_in[b][chunk][hl * P : (hl + 1) * P, qq * 512 : (qq + 1) * 512],
                    ctxn_t,
                )

            def all_gather(b, chunk):
                nc.gpsimd.collective_compute(
                    "AllGather", mybir.AluOpType.bypass, replica_groups=ccg,
                    ins=[cc_in[b][chunk].opt()], outs=[cc_out[b][chunk].opt()],
                )

            def all_gather13(h):
                nc.gpsimd.collective_compute(
                    "AllGather", mybir.AluOpType.bypass, replica_groups=ccg,
                    ins=[cc_in13[h].opt()], outs=[cc_out13[h].opt()],
                )

            with (
                tc.tile_pool(name="mask", bufs=1) as maskp,
                tc.tile_pool(name="qk1", bufs=1) as qk1p,
                tc.tile_pool(name="vt", bufs=1) as vtp,
                tc.tile_pool(name="v1", bufs=1) as v1p,
                tc.tile_pool(name="ow1", bufs=1) as ow1p,
            ):
                mask_sb = maskp.tile([P, 16, S], BF16)
                qk_sbs = [None, qk1p.tile([P, 2 * HPC, S], BF16, name="qksb1")]
                v_sbs = [None, v1p.tile([P, HPC, 16, P], F8, name="vsb1")]
                ow_sbs = [None, ow1p.tile([P, HPC, 16, P], F8, name="owsb1")]

                def attn_block(b, qc, hl, aps, attp, extra_mm):
                    """Attention for (b, head hl, q-chunk qc), k-tiles in
                    pairs; extra_mm(kp) emits extra matmuls per pair to keep
                    the PE busy while ACT/DVE run."""
                    qk = qk_sbs[b]
                    ctx_ps = aps.tile([P, 512], F32, tag="ctx", bufs=1)
                    sum_ps = aps.tile([P, 512], F32, tag="sum", bufs=1)
                    for kp in range(8):
                        kt0 = 2 * kp
                        s_ps = aps.tile([P, 1024], F32, tag="sco", bufs=2)
                        for u in range(2):
                            nc.tensor.matmul(
                                s_ps[:, u * 512 : (u + 1) * 512],
                                lhsT=qk[:, hl * 2 + 1, (kt0 + u) * P : (kt0 + u + 1) * P],
                                rhs=qk[:, hl * 2, qc * 512 : (qc + 1) * 512],
                                start=True,
                                stop=True,
                            )
                        exp_t = attp.tile([P, 1024], BF16, tag="exp")
                        nc.scalar.activation(exp_t, s_ps, AF.Exp, bias=esh_sb)
                        prob_t = attp.tile([P, 2, 512], F8, tag="prob")
                        nc.vector.tensor_mul(
                            prob_t,
                            exp_t.rearrange("p (u q) -> p u q", u=2),
                            mask_sb[:, kt0 : kt0 + 2, qc * 512 : (qc + 1) * 512],
                        )
                        # weave BEFORE ctx/sum: the PE queue is FIFO, so the
                        # independent woven matmuls must sit ahead of the
                        # prob-dependent ones to fill the exp/prob latency
                        extra_mm(kp)
                        nc.tensor.matmul(
                            ctx_ps,
                            lhsT=v_sbs[b][:, hl, kt0 : kt0 + 2, :],
                            rhs=prob_t,
                            start=(kp == 0),
                            stop=(kp == 7),
                            perf_mode=DR,
                        )
                        nc.tensor.matmul(
                            sum_ps,
                            lhsT=ow_sbs[b][:, hl, kt0 : kt0 + 2, :],
                            rhs=prob_t,
                            start=(kp == 0),
                            stop=(kp == 7),
                            perf_mode=DR,
                        )
                    rec_t = attp.tile([P, 512], F32, tag="rec", bufs=2)
                    nc.vector.reciprocal_approx_fast(rec_t, sum_ps)
                    ctxn_t = attp.tile([P, 512], F8, tag="ctxn", bufs=2)
                    nc.vector.tensor_mul(ctxn_t, ctx_ps, rec_t)
                    dma_ctx(b, qc, hl, ctxn_t)

                # ---------- phase 1: QKV(b0), standalone ----------
                with (
                    tc.tile_pool(name="qk0", bufs=1) as qk0p,
                    tc.tile_pool(name="v0", bufs=1) as v0p,
                    tc.tile_pool(name="ow0", bufs=1) as ow0p,
                    tc.tile_pool(name="wq", bufs=1) as wqp,
                    tc.tile_pool(name="hid", bufs=16) as hidp,
                    tc.tile_pool(name="qps", bufs=3, space="PSUM") as qps,
                ):
                    qk_sbs[0] = qk0p.tile([P, 2 * HPC, S], BF16, name="qksb0")
                    v_sbs[0] = v0p.tile([P, HPC, 16, P], F8, name="vsb0")
                    ow_sbs[0] = ow0p.tile([P, HPC, 16, P], F8, name="owsb0")
                    wq_sb = wqp.tile([P, 16, OSH], F8)

                    def qkv_sc(b, sc, vT_sb):
                        """QKV for one 512-wide s-chunk: 6 o-tiles x 8
                        h-tile-pairs (fp8 DoubleRow); call emit(j) for j in
                        range(48).  V^T o-tiles are PE-transposed to V [k, d]
                        and scaled by exp(alibi[k])*2^4 into fp8; ones' tiles
                        built alongside at 2^-1."""
                        hid_ts = []
                        for hp in range(8):
                            if b == 0 and sc == 0:
                                nc.gpsimd.dma_start(
                                    wq_sb[:, 2 * hp : 2 * hp + 2, :],
                                    wqkvT[2 * hp * P : (2 * hp + 2) * P, :].rearrange(
                                        "(two p) o -> p two o", p=P
                                    ),
                                )
                            hid_t = hidp.tile([P, 2, 512], F8, tag="hid")
                            nc.sync.dma_start(
                                hid_t,
                                hidT[
                                    2 * hp * P : (2 * hp + 2) * P,
                                    b * S + sc * 512 : b * S + (sc + 1) * 512,
                                ].rearrange("(two p) s -> p two s", p=P),
                            )
                            hid_ts.append(hid_t)
                        state = {"ps": None}

                        def emit(j):
                            ot, hp = divmod(j, 8)
                            hl, t = divmod(ot, 3)
                            if hp == 0:
                                state["ps"] = qps.tile(
                                    [P, 512], F32, tag="qkvps", bufs=2,
                                    name=f"qps_{b}_{sc}_{ot}",
                                )
                            nc.tensor.matmul(
                                state["ps"],
                                lhsT=wq_sb[:, 2 * hp : 2 * hp + 2, ot * P : (ot + 1) * P],
                                rhs=hid_ts[hp],
                                start=(hp == 0),
                                stop=(hp == 7),
                                perf_mode=DR,
                            )
                            if hp == 7:
                                # evacuate on DVE: keeps ScalarE exclusively on
                                # Exp (ACT table switches cost ~2.7us each).
                                # out = (psum + bias*2^10) * descale
                                dst = (
                                    vT_sb[:, hl, sc * 512 : (sc + 1) * 512]
                                    if t == 2
                                    else qk_sbs[b][:, hl * 2 + t, sc * 512 : (sc + 1) * 512]
                                )
                                desc = float(
                                    (INV if t == 0 else np.float32(1.0))
                                    * np.float32(2.0**-WSCALE)
                                )
                                nc.vector.tensor_scalar(
                                    dst, state["ps"],
                                    bq_sb[:, ot : ot + 1], desc,
                                    mybir.AluOpType.add, mybir.AluOpType.mult,
                                )
                                if t == 2:
                                    # V^T chunk ready: PE-transpose its 4
                                    # k-tiles (psum slots borrowed from the
                                    # qkv pool), scale rows into fp8
                                    for kk in range(4):
                                        kt = sc * 4 + kk
                                        acol = (b * HPC + hl) * 16 + kt
                                        vt_ps = qps.tile(
                                            [P, P], BF16, tag="qkvps", bufs=2,
                                            name=f"vt_{b}_{sc}_{hl}_{kk}",
                                        )
                                        nc.tensor.transpose(
                                            vt_ps,
                                            vT_sb[:, hl, kt * P : (kt + 1) * P],
                                            eye_sb,
                                        )
                                        nc.vector.tensor_scalar_mul(
                                            v_sbs[b][:, hl, kt, :],
                                            vt_ps,
                                            ale_sb[:, acol : acol + 1],
                                        )
                                        nc.vector.tensor_scalar_mul(
                                            ow_sbs[b][:, hl, kt, :],
                                            ones_sb,
                                            ale_sb[:, 64 + acol : 64 + acol + 1],
                                        )

                        return emit

                    vT0 = vtp.tile([P, HPC, S], BF16, tag="vT", name="vT0")
                    for sc in range(4):
                        emit = qkv_sc(0, sc, vT0)
                        for j in range(48):
                            emit(j)
                    # mask loads on the idle gpsimd SWDGE queues, deferred so
                    # they don't steal head bandwidth from wq/hid
                    for kt in range(16):
                        nc.gpsimd.dma_start(
                            mask_sb[:, kt, :], mask01T[kt * P : (kt + 1) * P, :]
                        )

                    # ---------- phase 2: attention(b0) + QKV(b1) ----------
                    with (
                        tc.tile_pool(name="att", bufs=3) as attp,
                        tc.tile_pool(name="aps", bufs=1, space="PSUM") as aps,
                    ):
                        vT1 = vtp.tile([P, HPC, S], BF16, tag="vT", name="vT1")
                        for qc in range(4):
                            for hl in range(HPC):
                                # 24 QKV(b1) matmuls woven into each block:
                                # 3 MMs per k-tile pair.
                                if hl == 0:
                                    emit = qkv_sc(1, qc, vT1)
                                base = 24 * hl

                                def extra(kp, emit=emit, base=base):
                                    for j in range(3):
                                        emit(base + kp * 3 + j)

                                attn_block(0, qc, hl, aps, attp, extra)
                            all_gather(0, qc)

                # ---------- phase 3: attention(b1) + dense(b0 + b1 early) --
                with (
                    tc.tile_pool(name="dw", bufs=1) as dwp,
                    tc.tile_pool(name="dctx", bufs=8) as dctxp,
                    tc.tile_pool(name="dps", bufs=2, space="PSUM") as dps,
                    tc.tile_pool(name="dout", bufs=3) as doutp,
                ):
                    wd_sb = dwp.tile([P, 16, DSH], F8)
                    nc.sync.dma_start(wd_sb, wdT.rearrange("(ht p) o -> p ht o", p=P))
                    # wd with h-tiles in [even heads | odd heads] order, for
                    # the tail chunk that consumes the per-head qc3 gathers
                    wd_sb_p = dwp.tile([P, 16, DSH], F8)
                    for t, ht in enumerate(list(range(0, 16, 2)) + list(range(1, 16, 2))):
                        nc.sync.dma_start(
                            wd_sb_p[:, t, :], wdT[ht * P : (ht + 1) * P, :]
                        )
                    rs_sb = dwp.tile([P, 2, B * S], F32)
                    nc.sync.dma_start(rs_sb, residT.rearrange("(ot p) s -> p ot s", p=P))

                    def dense_src(sc):
                        """cc_out chunk for output column chunk sc."""
                        return cc_out[sc // 4][sc % 4]

                    def dense_sc(sc):
                        """One 512-wide output column chunk: 2 o-tiles x 8
                        h-tile-pairs (fp8 DR); emit(j) for j in range(16)."""
                        src = dense_src(sc)
                        state = {}

                        def load(g):
                            # one 256KB DMA covers 2 h-tile pairs (4 h-tiles)
                            t = dctxp.tile([P, 4, 512], F8, tag="dctx", name="dctx_t")
                            nc.sync.dma_start(
                                t,
                                src[4 * g * P : (4 * g + 4) * P, :].rearrange(
                                    "(a p) q -> p a q", p=P
                                ),
                            )
                            state[f"c{g}"] = t

                        def emit(j):
                            hp, ot = divmod(j, 2)
                            if j == 0:
                                load(0)
                                load(1)
                            elif j in (4, 8):
                                load(j // 4 + 1)
                            state["ctx"] = state[f"c{hp // 2}"]
                            if hp == 0:
                                state[f"ps{ot}"] = dps.tile(
                                    [P, 512], F32, tag="dps", bufs=2,
                                    name=f"dps_{sc}_{ot}",
                                )
                            nc.tensor.matmul(
                                state[f"ps{ot}"],
                                lhsT=wd_sb[:, 2 * hp : 2 * hp + 2, ot * P : (ot + 1) * P],
                                rhs=state["ctx"][:, 2 * (hp % 2) : 2 * (hp % 2) + 2, :],
                                start=(hp == 0),
                                stop=(hp == 7),
                                perf_mode=DR,
                            )
                            if j == 15:
                                for o in range(2):
                                    o_t = doutp.tile([P, 512], F32, tag="o")
                                    nc.vector.tensor_add(
                                        o_t,
                                        state[f"ps{o}"],
                                        rs_sb[:, o, sc * 512 : (sc + 1) * 512],
                                    )
                                    nc.sync.dma_start(
                                        outT[o * P : (o + 1) * P, sc * 512 : (sc + 1) * 512],
                                        o_t,
                                    )

                        return emit

                    def dense_sc7():
                        """Tail chunk (b1 qc3 columns) against the per-head
                        gathers: h-tile pairs 0..3 = even heads (cc_out13[0]),
                        4..7 = odd heads (cc_out13[1]); wd_sb_p matches."""
                        state = {}

                        def load(g):
                            t = dctxp.tile([P, 4, 512], F8, tag="dctx", name="dctx_t")
                            src7 = cc_out13[g // 2]
                            r0 = (g % 2) * 4 * P
                            nc.sync.dma_start(
                                t,
                                src7[r0 : r0 + 4 * P, :].rearrange(
                                    "(a p) q -> p a q", p=P
                                ),
                            )
                            state[f"c{g}"] = t

                        def emit(j):
                            hp, ot = divmod(j, 2)
                            if j == 0:
                                load(0)
                                load(1)
                            elif j == 8:
                                load(2)
                                load(3)
                            if hp == 0:
                                state[f"ps{ot}"] = dps.tile(
                                    [P, 512], F32, tag="dps", bufs=2,
                                    name=f"dps_7_{ot}",
                                )
                            nc.tensor.matmul(
                                state[f"ps{ot}"],
                                lhsT=wd_sb_p[:, 2 * hp : 2 * hp + 2, ot * P : (ot + 1) * P],
                                rhs=state[f"c{hp // 2}"][:, 2 * (hp % 2) : 2 * (hp % 2) + 2, :],
                                start=(hp == 0),
                                stop=(hp == 7),
                                perf_mode=DR,
                            )
                            if j == 15:
                                for o in range(2):
                                    o_t = doutp.tile([P, 512], F32, tag="o")
                                    nc.vector.tensor_add(
                                        o_t,
                                        state[f"ps{o}"],
                                        rs_sb[:, o, 7 * 512 : 8 * 512],
                                    )
                                    nc.sync.dma_start(
                                        outT[o * P : (o + 1) * P, 7 * 512 : 8 * 512],
                                        o_t,
                                    )

                        return emit

                    with (
                        tc.tile_pool(name="att1", bufs=3) as attp,
                        tc.tile_pool(name="aps1", bufs=1, space="PSUM") as aps,
                    ):
                        # blocks 0..7 = (qc, hl); dense chunks sc0..sc6
                        # woven so each chunk's gather has landed before its
                        # block starts (b0 quarters land during phase 2; b1
                        # quarter qc lands ~1.5 blocks after block 2qc+1).
                        DENSE_AT = {0: [0], 1: [1], 2: [2], 4: [3], 5: [4], 6: [5], 7: [6]}
                        for qc in range(4):
                            for hl in range(HPC):
                                blk = qc * 2 + hl
                                if blk in DENSE_AT:
                                    emits = [dense_sc(s) for s in DENSE_AT[blk]]

                                    def extra(kp, emits=emits):
                                        for em in emits:
                                            for j in range(2):
                                                em(kp * 2 + j)
                                else:
                                    def extra(kp):
                                        pass
                                attn_block(1, qc, hl, aps, attp, extra)
                                if qc == 3:
                                    # per-head gather: hl0's half fires a
                                    # block early, hl1's is the (half-size)
                                    # tail gather
                                    all_gather13(hl)
                            if qc < 3:
                                all_gather(1, qc)

                    # ---------- phase 4: dense tail (last b1 column chunk) --
                    emit = dense_sc7()
                    for j in range(16):
                        emit(j)

    nc.compile()
    return nc


def _prep_in_maps(hidden_states, residual, alibi, attention_mask, w_qkv, b_qkv, w_dense, b_dense):
    f32 = np.float32
    wsc = f32(2.0**WSCALE)
    hs = np.asarray(hidden_states, f32).reshape(B * S, H)
    hidT = np.ascontiguousarray(hs.T).astype(NPF8)
    mask_keep = ~np.asarray(attention_mask).reshape(S, S)
    mask01T = np.ascontiguousarray(mask_keep.T).astype(NPBF16)
    ones_np = np.ones((P, P), f32).astype(NPBF16)
    al = np.asarray(alibi, f32).reshape(B, NH, S)
    resid = np.asarray(residual, f32).reshape(B * S, H)
    wq = np.asarray(w_qkv, f32)
    bq = np.asarray(b_qkv, f32)
    wd = np.asarray(w_dense, f32)
    bd = np.asarray(b_dense, f32)

    in_maps = []
    for r in range(NCORES):
        wshard = wq[r * OSH : (r + 1) * OSH] * wsc
        bshard = bq[r * OSH : (r + 1) * OSH] * wsc
        alcols_v = []
        alcols_o = []
        for b in range(B):
            for hl in range(HPC):
                e = np.exp(al[b, HPC * r + hl]).reshape(16, P).T
                alcols_v.append(e * f32(2.0**VSCALE))
                alcols_o.append(e * f32(2.0**OWSCALE))

        in_maps.append(
            {
                "hidT": hidT,
                "wqkvT": np.ascontiguousarray(wshard.T).astype(NPF8),
                "bqkv": np.ascontiguousarray(bshard.reshape(6, P).T),
                "mask01T": mask01T,
                "alibi_e": np.ascontiguousarray(
                    np.concatenate(alcols_v + alcols_o, axis=1)
                ),
                "wdT": np.ascontiguousarray(wd[r * DSH : (r + 1) * DSH].T * wsc).astype(NPF8),
                "residT": (
                    np.ascontiguousarray(resid[:, r * DSH : (r + 1) * DSH].T)
                    + bd[r * DSH : (r + 1) * DSH][:, None]
                ) / OUT_DESCALE,
                "ones": ones_np,
                "eye": np.eye(P, dtype=f32).astype(NPBF16),
            }
        )
    return in_maps


if os.environ.get("BASS_LDW_OPT"):
    _orig_run_command = bass_utils.run_command

    def _run_command_ldwopt(argv, **kwargs):
        argv = [
            "--enable-ldw-opt=true" if a == "--enable-ldw-opt=false" else a
            for a in argv
        ]
        return _orig_run_command(argv, **kwargs)

    bass_utils.run_command = _run_command_ldwopt


_NC_CACHE = {}


def run(inputs: dict, trace: bool = False):
    in_maps = _prep_in_maps(**inputs)
    if "nc" not in _NC_CACHE:
        _NC_CACHE["nc"] = build_nc()
    nc = _NC_CACHE["nc"]
    res = bass_utils.run_bass_kernel_spmd(
        nc, in_maps, core_ids=list(range(NCORES)), trace=trace
    )
    out = np.empty((B * S, H), np.float32)
    for r in range(NCORES):
        out[:, r * DSH : (r + 1) * DSH] = res.results[r]["outT"].T * OUT_DESCALE
    return out.reshape(B, S, H), res


def kernel(**inputs) -> np.ndarray:
    out, _ = run(inputs, trace=False)
    return out


# revision 21
# speedup vs baseline: 1.0748x; 1.0099x over previous
# Bloom parallel attention block on 8 trn2 NeuronCores, tensor-parallel over
# heads (2 heads per core).  Feature-major layouts throughout.
#
# fp8 (e4m3, max 240) + DoubleRow variant: QKV, ctx, softmax-denominator and
# dense matmuls run as K=256 fp8 DoubleRow matmuls (2x PE throughput); the
# score matmuls stay bf16 (K=128, no DR gain).  Power-of-2 scales keep fp8
# operands out of the subnormal range:
#   wqkv, wdense scaled 2^10 on host; descale folded into the QKV psum
#   evacuation (tensor_scalar mult) and the host-side output descale.
#   v' = v * exp(alibi) * 2^4,  ones' = exp(alibi) * 2^-1  =>  ctx = 2^5 ctx.
#   probs = exp(scores - 5.5) via ACT bias (global shift cancels in the
#   softmax normalization; keeps exp <= 240 for fp8e4).
#   dense out = 2^(5+10) * true; residual pre-scaled 2^15 on host, output
#   descaled 2^-15 on host.
# ctx is gathered in fp8 (halves collective bytes vs bf16).
#
# Per core r (heads 2r, 2r+1):
#   QKV DR matmul -> Q^T/K^T [d, s] bf16 and V^T [d, s] bf16 per batch.
#   V^T is PE-transposed to V [s, d] and scaled by exp(alibi[k])*2^4 into
#   fp8; ones' tiles = exp(alibi[k])*2^-1 broadcast columns (fp8).
#   attention (per b, head hl, 512-wide q-chunk qc), scores transposed [k, q]:
#     scores^T = K^T_tile.T @ Q^T       x2 k-tiles  (PE bf16, fp32 psum)
#     exp(scores - 5.5) over both k-tiles (ACT, bf16 out)
#     * mask01^T                        (DVE, 0/1 mask, fp8e4 out)
#     ctx^T += V'_pair.T @ probs        (PE fp8 DoubleRow, 4 k-tile pairs..)
#     sum   += ones'_pair.T @ probs     (PE fp8 DoubleRow, accum 8 pairs)
#     ctx^T *= 1/sum -> fp8 -> DMA to cc chunk
#   Pipelining: QKV(b1) matmuls are interleaved into attention(b0) k-loops,
#   dense matmuls into attention(b1) k-loops, so the PE never idles.  ctx
#   is AllGathered in 6 fp8 column chunks (2 for b0, 4 for b1).
#   dense: out^T[o_local, s] = wdT_pair.T @ ctx^T_full (fp8 DR) + resid'
#     (column-parallel => no all-reduce; host concatenates output slices)
import os
import sys

import numpy as np

if "/opt/trn_rl_repo" not in sys.path:
    sys.path.insert(0, "/opt/trn_rl_repo")

import ml_dtypes

import concourse.bass as bass
import concourse.mybir as mybir
import concourse.tile as tile
from concourse import bacc, bass_utils

B, S, H, NH = 2, 2048, 2048, 16
HD = H // NH            # 128
NCORES = 8
HPC = NH // NCORES      # heads per core = 2
OSH = 3 * H // NCORES   # qkv output rows per core = 768
DSH = H // NCORES       # dense output cols per core = 256
P = 128
F32 = mybir.dt.float32
BF16 = mybir.dt.bfloat16
F8 = mybir.dt.float8e4
AF = mybir.ActivationFunctionType
DR = mybir.MatmulPerfMode.DoubleRow
NPBF16 = ml_dtypes.bfloat16
NPF8 = ml_dtypes.float8_e4m3

WSCALE = 10          # wqkv/wdense host scale 2^10
VSCALE = 4           # v' = v*e*2^4
OWSCALE = -1         # ones' = e*2^-1  => ctxn = 2^5 ctx
ESHIFT = 5.5         # probs = exp(s - 5.5); cancels in normalization
OUT_DESCALE = np.float32(2.0 ** -(5 + WSCALE))   # host output descale
INV = np.float32(1.0 / np.sqrt(HD))


def build_nc():
    nc = bacc.Bacc(
        "TRN2",
        target_bir_lowering=False,
        debug=False,
        num_devices=NCORES,
    )

    hidT = nc.dram_tensor("hidT", [H, B * S], F8, kind="ExternalInput").ap()
    wqkvT = nc.dram_tensor("wqkvT", [H, OSH], F8, kind="ExternalInput").ap()
    bqkv = nc.dram_tensor("bqkv", [P, 6], F32, kind="ExternalInput").ap()
    mask01T = nc.dram_tensor("mask01T", [S, S], BF16, kind="ExternalInput").ap()
    alibi_e = nc.dram_tensor("alibi_e", [P, 2 * 2 * HPC * 16], F32, kind="ExternalInput").ap()
    wdT = nc.dram_tensor("wdT", [H, DSH], F8, kind="ExternalInput").ap()
    residT = nc.dram_tensor("residT", [DSH, B * S], F32, kind="ExternalInput").ap()
    ones = nc.dram_tensor("ones", [P, P], BF16, kind="ExternalInput").ap()
    eye = nc.dram_tensor("eye", [P, P], BF16, kind="ExternalInput").ap()
    outT = nc.dram_tensor("outT", [DSH, B * S], F32, kind="ExternalOutput").ap()

    with tile.TileContext(nc) as tc:
        ccg = [list(range(NCORES))]
        with (
            tc.tile_pool(name="const", bufs=1) as constp,
            tc.tile_pool(name="dram", bufs=1, space="DRAM") as dramp,
        ):
            bq_sb = constp.tile([P, 6], F32)
            nc.gpsimd.dma_start(bq_sb, bqkv)
            ale_sb = constp.tile([P, 2 * 2 * HPC * 16], F32)
            nc.gpsimd.dma_start(ale_sb, alibi_e)

            ones_sb = constp.tile(
                [P, P], BF16,
                name="ones_sb_ldw" if os.environ.get("BASS_LDW_OPT") else "ones_sb",
            )
            nc.gpsimd.dma_start(ones_sb, ones)
            eye_sb = constp.tile([P, P], BF16)
            nc.gpsimd.dma_start(eye_sb, eye)
            esh_sb = constp.tile([P, 1], F32)
            nc.gpsimd.memset(esh_sb, -ESHIFT)

            # ctx gather chunks (fp8): both batches in 4 column quarters,
            # each fired as soon as its q-chunk's two heads finish
            cc_spec = [(4, S // 4), (4, S // 4)]
            cc_in = [
                [
                    dramp.tile([HPC * HD, w], F8, name=f"cc_in{b}{i}")
                    for i in range(n)
                ]
                for b, (n, w) in enumerate(cc_spec)
            ]
            cc_out = [
                [
                    dramp.tile([H, w], F8, addr_space="Shared", name=f"cc_out{b}{i}")
                    for i in range(n)
                ]
                for b, (n, w) in enumerate(cc_spec)
            ]
            # b1 qc3 is gathered per-head so the first half fires one block
            # earlier and the tail gather is half-size.  Gathered row order
            # becomes [all even heads | all odd heads]; the tail dense chunk
            # uses a correspondingly permuted copy of wd.
            cc_in13 = [dramp.tile([P, 512], F8, name=f"cc_in13h{h}") for h in range(2)]
            cc_out13 = [
                dramp.tile([NCORES * P, 512], F8, addr_space="Shared", name=f"cc_out13h{h}")
                for h in range(2)
            ]

            def dma_ctx(b, qc, hl, ctxn_t):
                if b == 1 and qc == 3:
                    nc.sync.dma_start(cc_in13[hl], ctxn_t)
                    return
                n, w = cc_spec[b]
                chunk, qq = divmod(qc, 4 // n)
                nc.sync.dma_start(
                    cc_in[b][chunk][hl * P : (hl + 1) * P, qq * 512 : (qq + 1) * 512],
                    ctxn_t,
                )

            def all_gather(b, chunk):
                nc.gpsimd.collective_compute(
                    "AllGather", mybir.AluOpType.bypass, replica_groups=ccg,
                    ins=[cc_in[b][chunk].opt()], outs=[cc_out[b][chunk].opt()],
                )

            def all_gather13(h):
                nc.gpsimd.collective_compute(
                    "AllGather", mybir.AluOpType.bypass, replica_groups=ccg,
                    ins=[cc_in13[h].opt()], outs=[cc_out13[h].opt()],
                )

            with (
                tc.tile_pool(name="mask", bufs=1) as maskp,
                tc.tile_pool(name="qk1", bufs=1) as qk1p,
                tc.tile_pool(name="vt", bufs=1) as vtp,
                tc.tile_pool(name="v1", bufs=1) as v1p,
                tc.tile_pool(name="ow1", bufs=1) as ow1p,
            ):
                mask_sb = maskp.tile([P, 16, S], BF16)
                qk_sbs = [None, qk1p.tile([P, 2 * HPC, S], BF16, name="qksb1")]
                v_sbs = [None, v1p.tile([P, HPC, 16, P], F8, name="vsb1")]
                ow_sbs = [None, ow1p.tile([P, HPC, 16, P], F8, name="owsb1")]

                def attn_block(b, qc, hl, aps, attp, extra_mm):
                    """Attention for (b, head hl, q-chunk qc), k-tiles in
                    pairs; extra_mm(kp) emits extra matmuls per pair to keep
                    the PE busy while ACT/DVE run."""
                    qk = qk_sbs[b]
                    ctx_ps = aps.tile([P, 512], F32, tag="ctx", bufs=1)
                    sum_ps = aps.tile([P, 512], F32, tag="sum", bufs=1)
                    for kp in range(8):
                        kt0 = 2 * kp
                        s_ps = aps.tile([P, 1024], F32, tag="sco", bufs=2)
                        for u in range(2):
                            nc.tensor.matmul(
                                s_ps[:, u * 512 : (u + 1) * 512],
                                lhsT=qk[:, hl * 2 + 1, (kt0 + u) * P : (kt0 + u + 1) * P],
                                rhs=qk[:, hl * 2, qc * 512 : (qc + 1) * 512],
                                start=True,
                                stop=True,
                            )
                        exp_t = attp.tile([P, 1024], BF16, tag="exp", bufs=4)
                        nc.scalar.activation(exp_t, s_ps, AF.Exp, bias=esh_sb)
                        prob_t = attp.tile([P, 2, 512], F8, tag="prob", bufs=6)
                        nc.vector.tensor_mul(
                            prob_t,
                            exp_t.rearrange("p (u q) -> p u q", u=2),
                            mask_sb[:, kt0 : kt0 + 2, qc * 512 : (qc + 1) * 512],
                        )
                        # weave BEFORE ctx/sum: the PE queue is FIFO, so the
                        # independent woven matmuls must sit ahead of the
                        # prob-dependent ones to fill the exp/prob latency
                        extra_mm(kp)
                        nc.tensor.matmul(
                            ctx_ps,
                            lhsT=v_sbs[b][:, hl, kt0 : kt0 + 2, :],
                            rhs=prob_t,
                            start=(kp == 0),
                            stop=(kp == 7),
                            perf_mode=DR,
                        )
                        nc.tensor.matmul(
                            sum_ps,
                            lhsT=ow_sbs[b][:, hl, kt0 : kt0 + 2, :],
                            rhs=prob_t,
                            start=(kp == 0),
                            stop=(kp == 7),
                            perf_mode=DR,
                        )
                    rec_t = attp.tile([P, 512], F32, tag="rec", bufs=2)
                    nc.vector.reciprocal_approx_fast(rec_t, sum_ps)
                    ctxn_t = attp.tile([P, 512], F8, tag="ctxn", bufs=2)
                    nc.vector.tensor_mul(ctxn_t, ctx_ps, rec_t)
                    dma_ctx(b, qc, hl, ctxn_t)

                # ---------- phase 1: QKV(b0), standalone ----------
                with (
                    tc.tile_pool(name="qk0", bufs=1) as qk0p,
                    tc.tile_pool(name="v0", bufs=1) as v0p,
                    tc.tile_pool(name="ow0", bufs=1) as ow0p,
                    tc.tile_pool(name="wq", bufs=1) as wqp,
                    tc.tile_pool(name="hid", bufs=16) as hidp,
                    tc.tile_pool(name="qps", bufs=3, space="PSUM") as qps,
                ):
                    qk_sbs[0] = qk0p.tile([P, 2 * HPC, S], BF16, name="qksb0")
                    v_sbs[0] = v0p.tile([P, HPC, 16, P], F8, name="vsb0")
                    ow_sbs[0] = ow0p.tile([P, HPC, 16, P], F8, name="owsb0")
                    wq_sb = wqp.tile([P, 16, OSH], F8)

                    def qkv_sc(b, sc, vT_sb):
                        """QKV for one 512-wide s-chunk: 6 o-tiles x 8
                        h-tile-pairs (fp8 DoubleRow); call emit(j) for j in
                        range(48).  V^T o-tiles are PE-transposed to V [k, d]
                        and scaled by exp(alibi[k])*2^4 into fp8; ones' tiles
                        built alongside at 2^-1."""
                        hid_ts = []
                        for hp in range(8):
                            if b == 0 and sc == 0:
                                nc.gpsimd.dma_start(
                                    wq_sb[:, 2 * hp : 2 * hp + 2, :],
                                    wqkvT[2 * hp * P : (2 * hp + 2) * P, :].rearrange(
                                        "(two p) o -> p two o", p=P
                                    ),
                                )
                            hid_t = hidp.tile([P, 2, 512], F8, tag="hid")
                            nc.sync.dma_start(
                                hid_t,
                                hidT[
                                    2 * hp * P : (2 * hp + 2) * P,
                                    b * S + sc * 512 : b * S + (sc + 1) * 512,
                                ].rearrange("(two p) s -> p two s", p=P),
                            )
                            hid_ts.append(hid_t)
                        state = {"ps": None}

                        def emit(j):
                            ot, hp = divmod(j, 8)
                            hl, t = divmod(ot, 3)
                            if hp == 0:
                                state["ps"] = qps.tile(
                                    [P, 512], F32, tag="qkvps", bufs=2,
                                    name=f"qps_{b}_{sc}_{ot}",
                                )
                            nc.tensor.matmul(
                                state["ps"],
                                lhsT=wq_sb[:, 2 * hp : 2 * hp + 2, ot * P : (ot + 1) * P],
                                rhs=hid_ts[hp],
                                start=(hp == 0),
                                stop=(hp == 7),
                                perf_mode=DR,
                            )
                            if hp == 7:
                                # evacuate on DVE: keeps ScalarE exclusively on
                                # Exp (ACT table switches cost ~2.7us each).
                                # out = (psum + bias*2^10) * descale
                                dst = (
                                    vT_sb[:, hl, sc * 512 : (sc + 1) * 512]
                                    if t == 2
                                    else qk_sbs[b][:, hl * 2 + t, sc * 512 : (sc + 1) * 512]
                                )
                                desc = float(
                                    (INV if t == 0 else np.float32(1.0))
                                    * np.float32(2.0**-WSCALE)
                                )
                                nc.vector.tensor_scalar(
                                    dst, state["ps"],
                                    bq_sb[:, ot : ot + 1], desc,
                                    mybir.AluOpType.add, mybir.AluOpType.mult,
                                )
                                if t == 2:
                                    # V^T chunk ready: PE-transpose its 4
                                    # k-tiles (psum slots borrowed from the
                                    # qkv pool), scale rows into fp8
                                    for kk in range(4):
                                        kt = sc * 4 + kk
                                        acol = (b * HPC + hl) * 16 + kt
                                        vt_ps = qps.tile(
                                            [P, P], BF16, tag="qkvps", bufs=2,
                                            name=f"vt_{b}_{sc}_{hl}_{kk}",
                                        )
                                        nc.tensor.transpose(
                                            vt_ps,
                                            vT_sb[:, hl, kt * P : (kt + 1) * P],
                                            eye_sb,
                                        )
                                        nc.vector.tensor_scalar_mul(
                                            v_sbs[b][:, hl, kt, :],
                                            vt_ps,
                                            ale_sb[:, acol : acol + 1],
                                        )
                                        nc.vector.tensor_scalar_mul(
                                            ow_sbs[b][:, hl, kt, :],
                                            ones_sb,
                                            ale_sb[:, 64 + acol : 64 + acol + 1],
                                        )

                        return emit

                    vT0 = vtp.tile([P, HPC, S], BF16, tag="vT", name="vT0")
                    for sc in range(4):
                        emit = qkv_sc(0, sc, vT0)
                        for j in range(48):
                            emit(j)
                    # mask loads on the idle gpsimd SWDGE queues, deferred so
                    # they don't steal head bandwidth from wq/hid
                    for kt in range(16):
                        nc.gpsimd.dma_start(
                            mask_sb[:, kt, :], mask01T[kt * P : (kt + 1) * P, :]
                        )

                    # ---------- phase 2: attention(b0) + QKV(b1) ----------
                    with (
                        tc.tile_pool(name="att", bufs=3) as attp,
                        tc.tile_pool(name="aps", bufs=1, space="PSUM") as aps,
                    ):
                        vT1 = vtp.tile([P, HPC, S], BF16, tag="vT", name="vT1")
                        for qc in range(4):
                            for hl in range(HPC):
                                # 24 QKV(b1) matmuls woven into each block:
                                # 3 MMs per k-tile pair.
                                if hl == 0:
                                    emit = qkv_sc(1, qc, vT1)
                                base = 24 * hl

                                def extra(kp, emit=emit, base=base):
                                    for j in range(3):
                                        emit(base + kp * 3 + j)

                                attn_block(0, qc, hl, aps, attp, extra)
                            all_gather(0, qc)

                # ---------- phase 3: attention(b1) + dense(b0 + b1 early) --
                with (
                    tc.tile_pool(name="dw", bufs=1) as dwp,
                    tc.tile_pool(name="dctx", bufs=8) as dctxp,
                    tc.tile_pool(name="dps", bufs=2, space="PSUM") as dps,
                    tc.tile_pool(name="dout", bufs=3) as doutp,
                ):
                    wd_sb = dwp.tile([P, 16, DSH], F8)
                    nc.sync.dma_start(wd_sb, wdT.rearrange("(ht p) o -> p ht o", p=P))
                    # wd with h-tiles in [even heads | odd heads] order, for
                    # the tail chunk that consumes the per-head qc3 gathers
                    wd_sb_p = dwp.tile([P, 16, DSH], F8)
                    for t, ht in enumerate(list(range(0, 16, 2)) + list(range(1, 16, 2))):
                        nc.sync.dma_start(
                            wd_sb_p[:, t, :], wdT[ht * P : (ht + 1) * P, :]
                        )
                    rs_sb = dwp.tile([P, 2, B * S], F32)
                    nc.sync.dma_start(rs_sb, residT.rearrange("(ot p) s -> p ot s", p=P))

                    def dense_src(sc):
                        """cc_out chunk for output column chunk sc."""
                        return cc_out[sc // 4][sc % 4]

                    def dense_sc(sc):
                        """One 512-wide output column chunk: 2 o-tiles x 8
                        h-tile-pairs (fp8 DR); emit(j) for j in range(16)."""
                        src = dense_src(sc)
                        state = {}

                        def load(g):
                            # one 256KB DMA covers 2 h-tile pairs (4 h-tiles)
                            t = dctxp.tile([P, 4, 512], F8, tag="dctx", name="dctx_t")
                            nc.sync.dma_start(
                                t,
                                src[4 * g * P : (4 * g + 4) * P, :].rearrange(
                                    "(a p) q -> p a q", p=P
                                ),
                            )
                            state[f"c{g}"] = t

                        def loads():
                            for g in range(4):
                                load(g)

                        def emit(j):
                            hp, ot = divmod(j, 2)
                            state["ctx"] = state[f"c{hp // 2}"]
                            if hp == 0:
                                state[f"ps{ot}"] = dps.tile(
                                    [P, 512], F32, tag="dps", bufs=2,
                                    name=f"dps_{sc}_{ot}",
                                )
                            nc.tensor.matmul(
                                state[f"ps{ot}"],
                                lhsT=wd_sb[:, 2 * hp : 2 * hp + 2, ot * P : (ot + 1) * P],
                                rhs=state["ctx"][:, 2 * (hp % 2) : 2 * (hp % 2) + 2, :],
                                start=(hp == 0),
                                stop=(hp == 7),
                                perf_mode=DR,
                            )
                            if j == 15:
                                for o in range(2):
                                    o_t = doutp.tile([P, 512], F32, tag="o")
                                    nc.vector.tensor_add(
                                        o_t,
                                        state[f"ps{o}"],
                                        rs_sb[:, o, sc * 512 : (sc + 1) * 512],
                                    )
                                    nc.sync.dma_start(
                                        outT[o * P : (o + 1) * P, sc * 512 : (sc + 1) * 512],
                                        o_t,
                                    )

                        return loads, emit

                    def dense_sc7():
                        """Tail chunk (b1 qc3 columns) against the per-head
                        gathers: h-tile pairs 0..3 = even heads (cc_out13[0]),
                        4..7 = odd heads (cc_out13[1]); wd_sb_p matches."""
                        state = {}

                        def load(g):
                            t = dctxp.tile([P, 4, 512], F8, tag="dctx", name="dctx_t")
                            src7 = cc_out13[g // 2]
                            r0 = (g % 2) * 4 * P
                            nc.sync.dma_start(
                                t,
                                src7[r0 : r0 + 4 * P, :].rearrange(
                                    "(a p) q -> p a q", p=P
                                ),
                            )
                            state[f"c{g}"] = t

                        def emit(j):
                            hp, ot = divmod(j, 2)
                            if j == 0:
                                load(0)
                                load(1)
                            elif j == 8:
                                load(2)
                                load(3)
                            if hp == 0:
                                state[f"ps{ot}"] = dps.tile(
                                    [P, 512], F32, tag="dps", bufs=2,
                                    name=f"dps_7_{ot}",
                                )
                            nc.tensor.matmul(
                                state[f"ps{ot}"],
                                lhsT=wd_sb_p[:, 2 * hp : 2 * hp + 2, ot * P : (ot + 1) * P],
                                rhs=state[f"c{hp // 2}"][:, 2 * (hp % 2) : 2 * (hp % 2) + 2, :],
                                start=(hp == 0),
                                stop=(hp == 7),
                                perf_mode=DR,
                            )
                            if j == 15:
                                for o in range(2):
                                    o_t = doutp.tile([P, 512], F32, tag="o")
                                    nc.vector.tensor_add(
                                        o_t,
                                        state[f"ps{o}"],
                                        rs_sb[:, o, 7 * 512 : 8 * 512],
                                    )
                                    nc.sync.dma_start(
                                        outT[o * P : (o + 1) * P, 7 * 512 : 8 * 512],
                                        o_t,
                                    )

                        return emit

                    with (
                        tc.tile_pool(name="att1", bufs=3) as attp,
                        tc.tile_pool(name="aps1", bufs=1, space="PSUM") as aps,
                    ):
                        # blocks 0..7 = (qc, hl); dense chunks sc0..sc6
                        # woven so each chunk's gather has landed before its
                        # block starts (b0 quarters land during phase 2; b1
                        # quarter qc lands ~1.5 blocks after block 2qc+1).
                        DENSE_AT = {0: [0], 1: [1], 2: [2], 4: [3], 5: [4], 6: [5], 7: [6]}
                        for qc in range(4):
                            for hl in range(HPC):
                                blk = qc * 2 + hl
                                if blk in DENSE_AT:
                                    chunks = [dense_sc(s) for s in DENSE_AT[blk]]
                                    DSEG = [0, 3, 6, 9, 12, 14, 16]

                                    def extra(kp, chunks=chunks, DSEG=DSEG):
                                        for loads, em in chunks:
                                            if kp == 0:
                                                loads()
                                            elif kp >= 2:
                                                for j in range(DSEG[kp - 2], DSEG[kp - 1]):
                                                    em(j)
                                else:
                                    def extra(kp):
                                        pass
                                attn_block(1, qc, hl, aps, attp, extra)
                                if qc == 3:
                                    # per-head gather: hl0's half fires a
                                    # block early, hl1's is the (half-size)
                                    # tail gather
                                    all_gather13(hl)
                            if qc < 3:
                                all_gather(1, qc)

                    # ---------- phase 4: dense tail (last b1 column chunk) --
                    emit = dense_sc7()
                    for j in range(16):
                        emit(j)

    nc.compile()
    return nc


def _prep_in_maps(hidden_states, residual, alibi, attention_mask, w_qkv, b_qkv, w_dense, b_dense):
    f32 = np.float32
    wsc = f32(2.0**WSCALE)
    hs = np.asarray(hidden_states, f32).reshape(B * S, H)
    hidT = np.ascontiguousarray(hs.T).astype(NPF8)
    mask_keep = ~np.asarray(attention_mask).reshape(S, S)
    mask01T = np.ascontiguousarray(mask_keep.T).astype(NPBF16)
    ones_np = np.ones((P, P), f32).astype(NPBF16)
    al = np.asarray(alibi, f32).reshape(B, NH, S)
    resid = np.asarray(residual, f32).reshape(B * S, H)
    wq = np.asarray(w_qkv, f32)
    bq = np.asarray(b_qkv, f32)
    wd = np.asarray(w_dense, f32)
    bd = np.asarray(b_dense, f32)

    in_maps = []
    for r in range(NCORES):
        wshard = wq[r * OSH : (r + 1) * OSH] * wsc
        bshard = bq[r * OSH : (r + 1) * OSH] * wsc
        alcols_v = []
        alcols_o = []
        for b in range(B):
            for hl in range(HPC):
                e = np.exp(al[b, HPC * r + hl]).reshape(16, P).T
                alcols_v.append(e * f32(2.0**VSCALE))
                alcols_o.append(e * f32(2.0**OWSCALE))

        in_maps.append(
            {
                "hidT": hidT,
                "wqkvT": np.ascontiguousarray(wshard.T).astype(NPF8),
                "bqkv": np.ascontiguousarray(bshard.reshape(6, P).T),
                "mask01T": mask01T,
                "alibi_e": np.ascontiguousarray(
                    np.concatenate(alcols_v + alcols_o, axis=1)
                ),
                "wdT": np.ascontiguousarray(wd[r * DSH : (r + 1) * DSH].T * wsc).astype(NPF8),
                "residT": (
                    np.ascontiguousarray(resid[:, r * DSH : (r + 1) * DSH].T)
                    + bd[r * DSH : (r + 1) * DSH][:, None]
                ) / OUT_DESCALE,
                "ones": ones_np,
                "eye": np.eye(P, dtype=f32).astype(NPBF16),
            }
        )
    return in_maps


if os.environ.get("BASS_LDW_OPT"):
    _orig_run_command = bass_utils.run_command

    def _run_command_ldwopt(argv, **kwargs):
        argv = [
            "--enable-ldw-opt=true" if a == "--enable-ldw-opt=false" else a
            for a in argv
        ]
        return _orig_run_command(argv, **kwargs)

    bass_utils.run_command = _run_command_ldwopt


_NC_CACHE = {}


def run(inputs: dict, trace: bool = False):
    in_maps = _prep_in_maps(**inputs)
    if "nc" not in _NC_CACHE:
        _NC_CACHE["nc"] = build_nc()
    nc = _NC_CACHE["nc"]
    res = bass_utils.run_bass_kernel_spmd(
        nc, in_maps, core_ids=list(range(NCORES)), trace=trace
    )
    out = np.empty((B * S, H), np.float32)
    for r in range(NCORES):
        out[:, r * DSH : (r + 1) * DSH] = res.results[r]["outT"].T * OUT_DESCALE
    return out.reshape(B, S, H), res


def kernel(**inputs) -> np.ndarray:
    out, _ = run(inputs, trace=False)
    return out
